# revision 10
# baseline (speedup 1.0000x reference)
# Trainium2 Bass kernel for nn_CVXPolicy_MultiQuadcopter.
#
# Math (per sample):
#   x  = concat([t, z])                      (3073,)
#   h1 = tanh(x @ W1 + b1)                   (100,)
#   h2 = tanh(h1 @ W2 + b2)                  (100,)
#   p  = h2 @ W3 + b3                        (3072,)
#   c  = S(p)   (per-agent sparse linear map)   (1024,)
#   s  = ||c||^2 ; w = W(256*s) ; k = sqrt(256*w/s)
#   u* = -k * c
#
# Because c = S(p) is linear in p, S is folded into W3 on the host:
#   c = h2 @ (W3 @ S) + b3 @ S = h2 @ W3S + b3S
# which shrinks the last matmul 3x and removes all on-device shuffles.
#
# Sharding: pure data parallelism. Batch 8192 is split into 8 shards of
# 1024 rows, one per NeuronCore; the tiny MLP weights are replicated.
#
# Device pipeline per core (batch shard B=1024):
#   - z is cast-DMA'd (SWDGE) to bf16 on load; mm1 contracts over the
#     3072 dim, so z tiles are transposed on-chip through the PE
#     (identity matmul, bf16) into zT panels [128 d x 512 b] consumed as
#     the moving operand of mm1 (bf16, N=512 -> full PE rate + FWL
#     weight loads with 128-wide padded W1 chunks).
#   - Layer-1/2 activations are kept transposed ([feature, batch]); those
#     matmuls run in fp32r (cheap; N=512). b1/b2 are applied as
#     per-partition bias in the tanh activation op.
#   - mm3 produces c in natural layout [128 b x 1024]; b3S is added as a
#     host-prebroadcast [128, 1024] tile; squared row-sums give s;
#     Lambert-W runs via Halley iterations, pipelined in three slices
#     (tiles 0-3 / 4-5 / 6-7) so only the last slice's solve sits on the
#     critical-path tail; c is scaled by -k and streamed out.

import numpy as np
import ml_dtypes
from contextlib import ExitStack

import concourse.bass as bass
import concourse.tile as tile
from concourse import bacc, mybir
from concourse.bass_utils import run_bass_kernel_spmd

F32 = mybir.dt.float32
F32R = mybir.dt.float32r
BF16 = mybir.dt.bfloat16

N_CORES = 8
BATCH = 8192
B = BATCH // N_CORES      # batch rows per core
D = 3072                  # state dim
H = 100                   # hidden
CD = 1024                 # control dim
NCH = D // 128            # 24 contraction chunks for mm1
NBT = B // 128            # 8 batch tiles per core
GROUP = 512               # batch columns per mm1 pass
NG = B // GROUP           # 2 groups per core
TPG = GROUP // 128        # 4 batch tiles per group
MASS = 0.5
N_HALLEY = 5

AF = mybir.ActivationFunctionType
ALU = mybir.AluOpType


def build_kernel():
    nc = bacc.Bacc(None, target_bir_lowering=False, enable_partition_id=False)

    z_d = nc.declare_dram_parameter("z", [B, D], F32, isOutput=False)
    tT_d = nc.declare_dram_parameter("tT", [1, B], F32, isOutput=False)
    w1m_d = nc.declare_dram_parameter("w1m", [128, NCH * 128], BF16, isOutput=False)
    w1e_d = nc.declare_dram_parameter("w1e", [1, 128], BF16, isOutput=False)
    b1c_d = nc.declare_dram_parameter("b1c", [H, 1], F32, isOutput=False)
    w2_d = nc.declare_dram_parameter("w2", [H, H], F32R, isOutput=False)
    b2c_d = nc.declare_dram_parameter("b2c", [H, 1], F32, isOutput=False)
    w3s_d = nc.declare_dram_parameter("w3s", [H, CD], F32R, isOutput=False)
    b3f_d = nc.declare_dram_parameter("b3f", [128, CD], F32, isOutput=False)
    id_d = nc.declare_dram_parameter("ident", [128, 128], BF16, isOutput=False)
    out_d = nc.declare_dram_parameter("out", [B, CD], F32, isOutput=True)

    with ExitStack() as ctx:
        tc = ctx.enter_context(tile.TileContext(nc))

        const = ctx.enter_context(tc.tile_pool(name="const", bufs=1))
        zpool = ctx.enter_context(tc.tile_pool(name="znat", bufs=2 * TPG))
        ztp = ctx.enter_context(tc.tile_pool(name="zt", bufs=3))
        hpool = ctx.enter_context(tc.tile_pool(name="hs", bufs=2))
        cpool = ctx.enter_context(tc.tile_pool(name="call", bufs=1))
        opool = ctx.enter_context(tc.tile_pool(name="outs", bufs=2))
        sqpool = ctx.enter_context(tc.tile_pool(name="sq", bufs=2))
        lwp = ctx.enter_context(tc.tile_pool(name="lw", bufs=1))
        pt_ps = ctx.enter_context(tc.tile_pool(name="ptp", bufs=3, space="PSUM"))
        h1_ps = ctx.enter_context(tc.tile_pool(name="h1p", bufs=2, space="PSUM"))
        h2_ps = ctx.enter_context(tc.tile_pool(name="h2p", bufs=1, space="PSUM"))
        c_ps = ctx.enter_context(tc.tile_pool(name="cp", bufs=2, space="PSUM"))

        # ---- z loads for group 0 go out first (SWDGE, casting f32->bf16);
        # weight DMAs ride HWDGE queues in parallel.
        zn_group = {}
        for g in range(NG):
            zn_group[g] = []

        def load_group(g):
            for q in range(TPG):
                bt = TPG * g + q
                znt = zpool.tile([128, D], BF16, tag="zn", name="zn")
                if bt == 0:
                    # chunked so the first transposes can start early
                    for ck in range(3):
                        cs = ck * (D // 3)
                        nc.gpsimd.dma_start(
                            znt[:, cs:cs + D // 3],
                            z_d[bt * 128:(bt + 1) * 128, cs:cs + D // 3],
                        )
                else:
                    nc.gpsimd.dma_start(znt[:], z_d[bt * 128:(bt + 1) * 128, :])
                zn_group[g].append(znt)

        load_group(0)

        # ---- constants / weights ----
        ident = const.tile([128, 128], BF16, tag="ident")
        nc.sync.dma_start(ident[:], id_d[:])
        w1s = const.tile([128, NCH, 128], BF16, tag="w1s")
        nc.sync.dma_start(w1s[:], w1m_d[:].rearrange("p (c h) -> p c h", c=NCH))
        w1e = const.tile([1, 128], BF16, tag="w1e")
        nc.sync.dma_start(w1e[:], w1e_d[:])
        te = const.tile([1, B], BF16, tag="te")
        nc.gpsimd.dma_start(te[:], tT_d[:])
        b1c = const.tile([H, 1], F32, tag="b1c")
        nc.sync.dma_start(b1c[:], b1c_d[:])
        w2 = const.tile([H, H], F32R, tag="w2")
        nc.sync.dma_start(w2[:], w2_d[:])
        b2c = const.tile([H, 1], F32, tag="b2c")
        nc.sync.dma_start(b2c[:], b2c_d[:])
        w3s = const.tile([H, CD], F32R, tag="w3s")
        nc.sync.dma_start(w3s[:], w3s_d[:])
        b3f = const.tile([128, CD], F32, tag="b3f")
        nc.sync.dma_start(b3f[:], b3f_d[:])

        load_group(1)

        c_all = cpool.tile([128, NBT, CD], F32, tag="c_all")
        s_all = lwp.tile([128, NBT], F32, tag="s_all")

        def lambert_and_store(st, cnt):
            """Solve W for tiles [st, st+cnt) via asymptotic series + two
            log-Newton polish steps, scale c by -k, DMA out."""
            def lt(nm):
                return lwp.tile([128, cnt], F32, tag=f"{nm}{st}", name=f"{nm}{st}")

            sv = s_all[:, st:st + cnt]
            x = lt("lw_x")
            nc.vector.tensor_scalar(x[:], sv, 256.0, 8.0, ALU.mult, ALU.max)
            L1 = lt("lw_L1")
            nc.scalar.activation(L1[:], x[:], AF.Ln)
            L2 = lt("lw_L2")
            nc.scalar.activation(L2[:], L1[:], AF.Ln)
            # w = L1 - L2 + L2/L1 + L2*(L2-2)/(2*L1^2)
            r1 = lt("lw_r1")
            nc.vector.reciprocal(r1[:], L1[:])
            a = lt("lw_a")
            nc.vector.tensor_mul(a[:], L2[:], r1[:])
            w = lt("lw_w")
            nc.vector.tensor_sub(w[:], L1[:], L2[:])
            nc.vector.tensor_add(w[:], w[:], a[:])
            t = lt("lw_t")
            nc.vector.tensor_scalar(t[:], L2[:], -2.0, 0.5, ALU.add, ALU.mult)
            nc.vector.tensor_mul(t[:], t[:], a[:])
            nc.vector.tensor_mul(t[:], t[:], r1[:])
            nc.vector.tensor_add(w[:], w[:], t[:])
            # polish: w -= (ln w + w - ln x) * w / (1 + w)
            g = lt("lw_g")
            wp1 = lt("lw_wp1")
            rcp = lt("lw_rcp")
            for _ in range(2):
                nc.scalar.activation(g[:], w[:], AF.Ln)
                nc.vector.tensor_add(g[:], g[:], w[:])
                nc.vector.tensor_sub(g[:], g[:], L1[:])
                nc.vector.tensor_scalar_add(wp1[:], w[:], 1.0)
                nc.vector.reciprocal(rcp[:], wp1[:])
                nc.vector.tensor_mul(g[:], g[:], w[:])
                nc.vector.tensor_mul(g[:], g[:], rcp[:])
                nc.vector.tensor_sub(w[:], w[:], g[:])
            # kneg = -sqrt(256*w/s)  (0 when s == 0 since then w ~ W(8-guard)*0)
            sg = lt("lw_sg")
            nc.vector.tensor_scalar_max(sg[:], sv, 1e-30)
            nc.vector.reciprocal(rcp[:], sg[:])
            nc.vector.tensor_mul(sg[:], w[:], rcp[:])
            kneg = lt("lw_kneg")
            nc.scalar.activation(kneg[:], sg[:], AF.Sqrt, scale=256.0)
            nc.vector.tensor_scalar_mul(kneg[:], kneg[:], -1.0)
            for i in range(cnt):
                bt = st + i
                ot = opool.tile([128, CD], F32, tag="ot", name="ot")
                nc.vector.tensor_scalar_mul(ot[:], c_all[:, bt, :], kneg[:, i:i + 1])
                nc.sync.dma_start(out_d[bt * 128:(bt + 1) * 128, :], ot[:])

        # ---- main loop ----
        # b-tile-major: each batch tile's transposes+mm1 start as soon as
        # ITS z DMA lands; h1p accumulates all four quarters in one PSUM
        # group (the t-column matmul opens the group covering the full
        # bank, so quarter writes accumulate correctly). The tail stages
        # (tanh/mm2/tanh/mm3/c/s) run per-quarter so the last batch tile
        # has a short critical path, and Lambert solves go per tile-pair.
        NJG = NCH // 8  # 3 pt-panels of 8 d-chunks per b-tile

        def tail_q(g, q, h1p):
            bt = TPG * g + q
            h1s = hpool.tile([H, 128], F32R, tag="h1s", name="h1s")
            nc.scalar.activation(h1s[:], h1p[0:H, :], AF.Tanh, bias=b1c[:])
            h2p = h2_ps.tile([H, 128], F32, tag="h2p", name="h2p")
            nc.tensor.matmul(h2p[:], w2[:], h1s[:], start=True, stop=True)
            h2s = hpool.tile([H, 128], F32R, tag="h2s", name="h2s")
            nc.scalar.activation(h2s[:], h2p[:], AF.Tanh, bias=b2c[:])
            for nb in range(2):
                cp = c_ps.tile([128, 512], F32, tag="cp", name="cp")
                nc.tensor.matmul(
                    cp[:], h2s[:], w3s[:, nb * 512:(nb + 1) * 512],
                    start=True, stop=True,
                )
                # c = cp + b3S  (DVE, PSUM -> SBUF)
                nc.vector.tensor_add(
                    c_all[:, bt, nb * 512:(nb + 1) * 512],
                    cp[:], b3f[:, nb * 512:(nb + 1) * 512],
                )
            sq = sqpool.tile([128, CD], F32, tag="sq", name="sq")
            nc.scalar.activation(
                sq[:], c_all[:, bt, :], AF.Square,
                accum_out=s_all[:, bt:bt + 1],
            )

        for g in range(NG):
            zn = zn_group[g]
            work = [(q, jg) for q in range(TPG) for jg in range(NJG)]
            h1ps = {}
            pts = {}

            def emit_transpose(idx):
                q, jg = work[idx]
                pt = pt_ps.tile([128, 1024], BF16, tag="pt", name="pt")
                for u in range(8):
                    j = jg * 8 + u
                    nc.tensor.matmul(
                        pt[:, u * 128:(u + 1) * 128],
                        zn[q][:, j * 128:(j + 1) * 128],
                        ident[:],
                        start=(u == 0), stop=(u == 7),
                        is_transpose=True,
                    )
                pts[idx] = pt

            emit_transpose(0)
            for idx, (q, jg) in enumerate(work):
                if idx + 1 < len(work):
                    emit_transpose(idx + 1)  # keep PE one panel ahead
                zt = ztp.tile([128, 1024], BF16, tag="zt", name="zt")
                if idx % 3 == 2:
                    nc.scalar.copy(zt[:], pts.pop(idx)[:])
                else:
                    nc.vector.tensor_copy(zt[:], pts.pop(idx)[:])
                if jg == 0:
                    bt = TPG * g + q
                    h1ps[q] = h1_ps.tile([128, 128], F32, tag="h1p", name="h1p")
                    # t column opens the accumulation group: h1 += t*W1[0,:]
                    nc.tensor.matmul(
                        h1ps[q][:], w1e[:], te[:, bt * 128:(bt + 1) * 128],
                        start=True, stop=False,
                    )
                for u in range(8):
                    j = jg * 8 + u
                    nc.tensor.matmul(
                        h1ps[q][:],
                        w1s[:, j, :], zt[:, u * 128:(u + 1) * 128],
                        start=False, stop=(jg == NJG - 1 and u == 7),
                    )
                if jg == NJG - 1:
                    tail_q(g, q, h1ps.pop(q))
                    bt = TPG * g + q
                    if bt % 2 == 1:
                        lambert_and_store(bt - 1, 2)

    nc.compile()
    return nc


def host_prep(z, t, W1, b1, W2, b2, W3, b3):
    """Host-side weight re-layout + per-core shard maps."""
    f = np.float32
    bf = ml_dtypes.bfloat16
    z = np.asarray(z, f)
    t = np.asarray(t, f)
    W1 = np.asarray(W1, f)
    b1 = np.asarray(b1, f)
    W2 = np.asarray(W2, f)
    b2 = np.asarray(b2, f)
    W3 = np.asarray(W3, f)
    b3 = np.asarray(b3, f)

    # mm1 stationary chunks (bf16, padded to 128 cols for FWL):
    # w1m[p, j*128 + h] = W1[1 + j*128 + p, h]
    w1m = np.zeros((128, NCH, 128), bf)
    w1m[:, :, :H] = W1[1:, :].reshape(NCH, 128, H).transpose(1, 0, 2).astype(bf)
    w1m = np.ascontiguousarray(w1m.reshape(128, NCH * 128))
    w1e = np.zeros((1, 128), bf)
    w1e[0, :H] = W1[0, :].astype(bf)
    b1c = np.ascontiguousarray(b1.reshape(H, 1))
    b2c = np.ascontiguousarray(b2.reshape(H, 1))

    # fold the p -> c map into W3 (and b3)
    W3r = W3.reshape(H, CD // 4, 12)
    W3S = np.empty((H, CD // 4, 4), f)
    W3S[..., 0] = (W3r[..., 6] + W3r[..., 7] + W3r[..., 8]) / MASS
    W3S[..., 1] = W3r[..., 9]
    W3S[..., 2] = W3r[..., 10]
    W3S[..., 3] = W3r[..., 11]
    b3r = b3.reshape(CD // 4, 12)
    b3S = np.empty((CD // 4, 4), f)
    b3S[..., 0] = (b3r[..., 6] + b3r[..., 7] + b3r[..., 8]) / MASS
    b3S[..., 1] = b3r[..., 9]
    b3S[..., 2] = b3r[..., 10]
    b3S[..., 3] = b3r[..., 11]
    w3s = np.ascontiguousarray(W3S.reshape(H, CD))
    b3f = np.ascontiguousarray(np.broadcast_to(b3S.reshape(1, CD), (128, CD)))

    ident = np.eye(128, dtype=bf)

    in_maps = []
    for c in range(N_CORES):
        sl = slice(c * B, (c + 1) * B)
        in_maps.append({
            "z": np.ascontiguousarray(z[sl]),
            "tT": np.ascontiguousarray(t[sl].reshape(1, B)),
            "w1m": w1m,
            "w1e": w1e,
            "b1c": b1c,
            "w2": W2,
            "b2c": b2c,
            "w3s": w3s,
            "b3f": b3f,
            "ident": ident,
        })
    return in_maps


_NC_CACHE = None


def _get_nc():
    global _NC_CACHE
    if _NC_CACHE is None:
        _NC_CACHE = build_kernel()
    return _NC_CACHE


def run(inputs, trace=False):
    """Returns (full_output, BassKernelResults)."""
    nc = _get_nc()
    in_maps = host_prep(**inputs)
    res = run_bass_kernel_spmd(
        nc, in_maps, list(range(N_CORES)), trace=trace,
    )
    out = np.concatenate([r["out"] for r in res.results], axis=0)
    return out.astype(np.float32, copy=False), res


def kernel(**inputs):
    out, _ = run(inputs)
    return out


# revision 11
# speedup vs baseline: 1.0229x; 1.0229x over previous
# Trainium2 Bass kernel for nn_CVXPolicy_MultiQuadcopter.
#
# Math (per sample):
#   x  = concat([t, z])                      (3073,)
#   h1 = tanh(x @ W1 + b1)                   (100,)
#   h2 = tanh(h1 @ W2 + b2)                  (100,)
#   p  = h2 @ W3 + b3                        (3072,)
#   c  = S(p)   (per-agent sparse linear map)   (1024,)
#   s  = ||c||^2 ; w = W(256*s) ; k = sqrt(256*w/s)
#   u* = -k * c
#
# Because c = S(p) is linear in p, S is folded into W3 on the host:
#   c = h2 @ (W3 @ S) + b3 @ S = h2 @ W3S + b3S
# which shrinks the last matmul 3x and removes all on-device shuffles.
#
# Sharding: pure data parallelism. Batch 8192 is split into 8 shards of
# 1024 rows, one per NeuronCore; the tiny MLP weights are replicated.
#
# Device pipeline per core (batch shard B=1024):
#   - z is cast-DMA'd (SWDGE) to bf16 on load; mm1 contracts over the
#     3072 dim, so z tiles are transposed on-chip through the PE
#     (identity matmul, bf16) into zT panels [128 d x 512 b] consumed as
#     the moving operand of mm1 (bf16, N=512 -> full PE rate + FWL
#     weight loads with 128-wide padded W1 chunks).
#   - Layer-1/2 activations are kept transposed ([feature, batch]); those
#     matmuls run in fp32r (cheap; N=512). b1/b2 are applied as
#     per-partition bias in the tanh activation op.
#   - mm3 produces c in natural layout [128 b x 1024]; b3S is added as a
#     host-prebroadcast [128, 1024] tile; squared row-sums give s;
#     Lambert-W runs via Halley iterations, pipelined in three slices
#     (tiles 0-3 / 4-5 / 6-7) so only the last slice's solve sits on the
#     critical-path tail; c is scaled by -k and streamed out.

import numpy as np
import ml_dtypes
from contextlib import ExitStack

import concourse.bass as bass
import concourse.tile as tile
from concourse import bacc, mybir
from concourse.bass_utils import run_bass_kernel_spmd

F32 = mybir.dt.float32
F32R = mybir.dt.float32r
BF16 = mybir.dt.bfloat16

N_CORES = 8
BATCH = 8192
B = BATCH // N_CORES      # batch rows per core
D = 3072                  # state dim
H = 100                   # hidden
CD = 1024                 # control dim
NCH = D // 128            # 24 contraction chunks for mm1
NBT = B // 128            # 8 batch tiles per core
GROUP = 512               # batch columns per mm1 pass
NG = B // GROUP           # 2 groups per core
TPG = GROUP // 128        # 4 batch tiles per group
MASS = 0.5
N_HALLEY = 5

AF = mybir.ActivationFunctionType
ALU = mybir.AluOpType


def build_kernel():
    nc = bacc.Bacc(None, target_bir_lowering=False, enable_partition_id=False)

    z_d = nc.declare_dram_parameter("z", [B, D], F32, isOutput=False)
    tT_d = nc.declare_dram_parameter("tT", [1, B], F32, isOutput=False)
    w1m_d = nc.declare_dram_parameter("w1m", [128, NCH * 128], BF16, isOutput=False)
    w1e_d = nc.declare_dram_parameter("w1e", [1, 128], BF16, isOutput=False)
    b1c_d = nc.declare_dram_parameter("b1c", [H, 1], F32, isOutput=False)
    w2_d = nc.declare_dram_parameter("w2", [H, H], F32R, isOutput=False)
    b2c_d = nc.declare_dram_parameter("b2c", [H, 1], F32, isOutput=False)
    w3s_d = nc.declare_dram_parameter("w3s", [H, CD], F32R, isOutput=False)
    b3f_d = nc.declare_dram_parameter("b3f", [128, CD], F32, isOutput=False)
    id_d = nc.declare_dram_parameter("ident", [128, 128], BF16, isOutput=False)
    out_d = nc.declare_dram_parameter("out", [B, CD], F32, isOutput=True)

    with ExitStack() as ctx:
        tc = ctx.enter_context(tile.TileContext(nc))

        const = ctx.enter_context(tc.tile_pool(name="const", bufs=1))
        zpool = ctx.enter_context(tc.tile_pool(name="znat", bufs=2 * TPG))
        ztp = ctx.enter_context(tc.tile_pool(name="zt", bufs=3))
        hpool = ctx.enter_context(tc.tile_pool(name="hs", bufs=2))
        cpool = ctx.enter_context(tc.tile_pool(name="call", bufs=1))
        opool = ctx.enter_context(tc.tile_pool(name="outs", bufs=2))
        sqpool = ctx.enter_context(tc.tile_pool(name="sq", bufs=2))
        lwp = ctx.enter_context(tc.tile_pool(name="lw", bufs=1))
        pt_ps = ctx.enter_context(tc.tile_pool(name="ptp", bufs=3, space="PSUM"))
        h1_ps = ctx.enter_context(tc.tile_pool(name="h1p", bufs=2, space="PSUM"))
        h2_ps = ctx.enter_context(tc.tile_pool(name="h2p", bufs=1, space="PSUM"))
        c_ps = ctx.enter_context(tc.tile_pool(name="cp", bufs=2, space="PSUM"))

        # ---- z loads for group 0 go out first (SWDGE, casting f32->bf16);
        # weight DMAs ride HWDGE queues in parallel.
        zn_group = {}
        for g in range(NG):
            zn_group[g] = []

        def load_group(g):
            for q in range(TPG):
                bt = TPG * g + q
                znt = zpool.tile([128, D], BF16, tag="zn", name="zn")
                if bt == 0:
                    # chunked so the first transposes can start early
                    for ck in range(3):
                        cs = ck * (D // 3)
                        nc.gpsimd.dma_start(
                            znt[:, cs:cs + D // 3],
                            z_d[bt * 128:(bt + 1) * 128, cs:cs + D // 3],
                        )
                else:
                    nc.gpsimd.dma_start(znt[:], z_d[bt * 128:(bt + 1) * 128, :])
                zn_group[g].append(znt)

        load_group(0)

        # ---- constants / weights ----
        ident = const.tile([128, 128], BF16, tag="ident")
        nc.sync.dma_start(ident[:], id_d[:])
        w1s = const.tile([128, NCH, 128], BF16, tag="w1s")
        nc.sync.dma_start(w1s[:], w1m_d[:].rearrange("p (c h) -> p c h", c=NCH))
        w1e = const.tile([1, 128], BF16, tag="w1e")
        nc.sync.dma_start(w1e[:], w1e_d[:])
        te = const.tile([1, B], BF16, tag="te")
        nc.gpsimd.dma_start(te[:], tT_d[:])
        b1c = const.tile([H, 1], F32, tag="b1c")
        nc.sync.dma_start(b1c[:], b1c_d[:])
        w2 = const.tile([H, H], F32R, tag="w2")
        nc.sync.dma_start(w2[:], w2_d[:])
        b2c = const.tile([H, 1], F32, tag="b2c")
        nc.sync.dma_start(b2c[:], b2c_d[:])
        w3s = const.tile([H, CD], F32R, tag="w3s")
        nc.sync.dma_start(w3s[:], w3s_d[:])
        b3f = const.tile([128, CD], F32, tag="b3f")
        nc.sync.dma_start(b3f[:], b3f_d[:])

        load_group(1)

        c_all = cpool.tile([128, NBT, CD], F32, tag="c_all")
        s_all = lwp.tile([128, NBT], F32, tag="s_all")

        def lambert_and_store(st, cnt):
            """Solve W for tiles [st, st+cnt) via asymptotic series + two
            log-Newton polish steps, scale c by -k, DMA out."""
            def lt(nm):
                return lwp.tile([128, cnt], F32, tag=f"{nm}{st}", name=f"{nm}{st}")

            for i in range(cnt):
                bt = st + i
                sq = sqpool.tile([128, CD], F32, tag="sq", name="sq")
                nc.scalar.activation(
                    sq[:], c_all[:, bt, :], AF.Square,
                    accum_out=s_all[:, bt:bt + 1],
                )
            sv = s_all[:, st:st + cnt]
            x = lt("lw_x")
            nc.vector.tensor_scalar(x[:], sv, 256.0, 8.0, ALU.mult, ALU.max)
            L1 = lt("lw_L1")
            nc.scalar.activation(L1[:], x[:], AF.Ln)
            L2 = lt("lw_L2")
            nc.scalar.activation(L2[:], L1[:], AF.Ln)
            # w = L1 - L2 + L2/L1 + L2*(L2-2)/(2*L1^2)
            r1 = lt("lw_r1")
            nc.vector.reciprocal(r1[:], L1[:])
            a = lt("lw_a")
            nc.vector.tensor_mul(a[:], L2[:], r1[:])
            w = lt("lw_w")
            nc.vector.tensor_sub(w[:], L1[:], L2[:])
            nc.vector.tensor_add(w[:], w[:], a[:])
            t = lt("lw_t")
            nc.vector.tensor_scalar(t[:], L2[:], -2.0, 0.5, ALU.add, ALU.mult)
            nc.vector.tensor_mul(t[:], t[:], a[:])
            nc.vector.tensor_mul(t[:], t[:], r1[:])
            nc.vector.tensor_add(w[:], w[:], t[:])
            # polish: w -= (ln w + w - ln x) * w / (1 + w)
            g = lt("lw_g")
            wp1 = lt("lw_wp1")
            rcp = lt("lw_rcp")
            for _ in range(2):
                nc.scalar.activation(g[:], w[:], AF.Ln)
                nc.vector.tensor_add(g[:], g[:], w[:])
                nc.vector.tensor_sub(g[:], g[:], L1[:])
                nc.vector.tensor_scalar_add(wp1[:], w[:], 1.0)
                nc.vector.reciprocal(rcp[:], wp1[:])
                nc.vector.tensor_mul(g[:], g[:], w[:])
                nc.vector.tensor_mul(g[:], g[:], rcp[:])
                nc.vector.tensor_sub(w[:], w[:], g[:])
            # kneg = -sqrt(256*w/s)  (0 when s == 0 since then w ~ W(8-guard)*0)
            sg = lt("lw_sg")
            nc.vector.tensor_scalar_max(sg[:], sv, 1e-30)
            nc.vector.reciprocal(rcp[:], sg[:])
            nc.vector.tensor_mul(sg[:], w[:], rcp[:])
            kneg = lt("lw_kneg")
            nc.scalar.activation(kneg[:], sg[:], AF.Sqrt, scale=256.0)
            nc.vector.tensor_scalar_mul(kneg[:], kneg[:], -1.0)
            for i in range(cnt):
                bt = st + i
                ot = opool.tile([128, CD], F32, tag="ot", name="ot")
                nc.scalar.mul(ot[:], c_all[:, bt, :], kneg[:, i:i + 1])
                nc.sync.dma_start(out_d[bt * 128:(bt + 1) * 128, :], ot[:])

        # ---- main loop ----
        # b-tile-major: each batch tile's transposes+mm1 start as soon as
        # ITS z DMA lands; h1p accumulates all four quarters in one PSUM
        # group (the t-column matmul opens the group covering the full
        # bank, so quarter writes accumulate correctly). The tail stages
        # (tanh/mm2/tanh/mm3/c/s) run per-quarter so the last batch tile
        # has a short critical path, and Lambert solves go per tile-pair.
        NJG = NCH // 8  # 3 pt-panels of 8 d-chunks per b-tile

        def tail_q(g, q, h1p):
            bt = TPG * g + q
            h1s = hpool.tile([H, 128], F32R, tag="h1s", name="h1s")
            nc.scalar.activation(h1s[:], h1p[0:H, :], AF.Tanh, bias=b1c[:])
            h2p = h2_ps.tile([H, 128], F32, tag="h2p", name="h2p")
            nc.tensor.matmul(h2p[:], w2[:], h1s[:], start=True, stop=True)
            h2s = hpool.tile([H, 128], F32R, tag="h2s", name="h2s")
            nc.scalar.activation(h2s[:], h2p[:], AF.Tanh, bias=b2c[:])
            for nb in range(2):
                cp = c_ps.tile([128, 512], F32, tag="cp", name="cp")
                nc.tensor.matmul(
                    cp[:], h2s[:], w3s[:, nb * 512:(nb + 1) * 512],
                    start=True, stop=True,
                )
                # c = cp + b3S  (DVE, PSUM -> SBUF)
                nc.vector.tensor_add(
                    c_all[:, bt, nb * 512:(nb + 1) * 512],
                    cp[:], b3f[:, nb * 512:(nb + 1) * 512],
                )

        for g in range(NG):
            zn = zn_group[g]
            work = [(q, jg) for q in range(TPG) for jg in range(NJG)]
            h1ps = {}
            pts = {}

            def emit_transpose(idx):
                q, jg = work[idx]
                pt = pt_ps.tile([128, 1024], BF16, tag="pt", name="pt")
                for u in range(8):
                    j = jg * 8 + u
                    nc.tensor.matmul(
                        pt[:, u * 128:(u + 1) * 128],
                        zn[q][:, j * 128:(j + 1) * 128],
                        ident[:],
                        start=(u == 0), stop=(u == 7),
                        is_transpose=True,
                    )
                pts[idx] = pt

            emit_transpose(0)
            for idx, (q, jg) in enumerate(work):
                if idx + 1 < len(work):
                    emit_transpose(idx + 1)  # keep PE one panel ahead
                zt = ztp.tile([128, 1024], BF16, tag="zt", name="zt")
                nc.vector.tensor_copy(zt[:], pts.pop(idx)[:])
                if jg == 0:
                    bt = TPG * g + q
                    h1ps[q] = h1_ps.tile([128, 128], F32, tag="h1p", name="h1p")
                    # t column opens the accumulation group: h1 += t*W1[0,:]
                    nc.tensor.matmul(
                        h1ps[q][:], w1e[:], te[:, bt * 128:(bt + 1) * 128],
                        start=True, stop=False,
                    )
                for u in range(8):
                    j = jg * 8 + u
                    nc.tensor.matmul(
                        h1ps[q][:],
                        w1s[:, j, :], zt[:, u * 128:(u + 1) * 128],
                        start=False, stop=(jg == NJG - 1 and u == 7),
                    )
                if jg == NJG - 1:
                    tail_q(g, q, h1ps.pop(q))
                    bt = TPG * g + q
                    if bt % 2 == 1:
                        lambert_and_store(bt - 1, 2)

    nc.compile()
    return nc


def host_prep(z, t, W1, b1, W2, b2, W3, b3):
    """Host-side weight re-layout + per-core shard maps."""
    f = np.float32
    bf = ml_dtypes.bfloat16
    z = np.asarray(z, f)
    t = np.asarray(t, f)
    W1 = np.asarray(W1, f)
    b1 = np.asarray(b1, f)
    W2 = np.asarray(W2, f)
    b2 = np.asarray(b2, f)
    W3 = np.asarray(W3, f)
    b3 = np.asarray(b3, f)

    # mm1 stationary chunks (bf16, padded to 128 cols for FWL):
    # w1m[p, j*128 + h] = W1[1 + j*128 + p, h]
    w1m = np.zeros((128, NCH, 128), bf)
    w1m[:, :, :H] = W1[1:, :].reshape(NCH, 128, H).transpose(1, 0, 2).astype(bf)
    w1m = np.ascontiguousarray(w1m.reshape(128, NCH * 128))
    w1e = np.zeros((1, 128), bf)
    w1e[0, :H] = W1[0, :].astype(bf)
    b1c = np.ascontiguousarray(b1.reshape(H, 1))
    b2c = np.ascontiguousarray(b2.reshape(H, 1))

    # fold the p -> c map into W3 (and b3)
    W3r = W3.reshape(H, CD // 4, 12)
    W3S = np.empty((H, CD // 4, 4), f)
    W3S[..., 0] = (W3r[..., 6] + W3r[..., 7] + W3r[..., 8]) / MASS
    W3S[..., 1] = W3r[..., 9]
    W3S[..., 2] = W3r[..., 10]
    W3S[..., 3] = W3r[..., 11]
    b3r = b3.reshape(CD // 4, 12)
    b3S = np.empty((CD // 4, 4), f)
    b3S[..., 0] = (b3r[..., 6] + b3r[..., 7] + b3r[..., 8]) / MASS
    b3S[..., 1] = b3r[..., 9]
    b3S[..., 2] = b3r[..., 10]
    b3S[..., 3] = b3r[..., 11]
    w3s = np.ascontiguousarray(W3S.reshape(H, CD))
    b3f = np.ascontiguousarray(np.broadcast_to(b3S.reshape(1, CD), (128, CD)))

    ident = np.eye(128, dtype=bf)

    in_maps = []
    for c in range(N_CORES):
        sl = slice(c * B, (c + 1) * B)
        in_maps.append({
            "z": np.ascontiguousarray(z[sl]),
            "tT": np.ascontiguousarray(t[sl].reshape(1, B)),
            "w1m": w1m,
            "w1e": w1e,
            "b1c": b1c,
            "w2": W2,
            "b2c": b2c,
            "w3s": w3s,
            "b3f": b3f,
            "ident": ident,
        })
    return in_maps


_NC_CACHE = None


def _get_nc():
    global _NC_CACHE
    if _NC_CACHE is None:
        _NC_CACHE = build_kernel()
    return _NC_CACHE


def run(inputs, trace=False):
    """Returns (full_output, BassKernelResults)."""
    nc = _get_nc()
    in_maps = host_prep(**inputs)
    res = run_bass_kernel_spmd(
        nc, in_maps, list(range(N_CORES)), trace=trace,
    )
    out = np.concatenate([r["out"] for r in res.results], axis=0)
    return out.astype(np.float32, copy=False), res


def kernel(**inputs):
    out, _ = run(inputs)
    return out


# revision 12
# speedup vs baseline: 1.1131x; 1.0882x over previous
# Trainium2 Bass kernel for nn_CVXPolicy_MultiQuadcopter.
#
# Math (per sample):
#   x  = concat([t, z])                      (3073,)
#   h1 = tanh(x @ W1 + b1)                   (100,)
#   h2 = tanh(h1 @ W2 + b2)                  (100,)
#   p  = h2 @ W3 + b3                        (3072,)
#   c  = S(p)   (per-agent sparse linear map)   (1024,)
#   s  = ||c||^2 ; w = W(256*s) ; k = sqrt(256*w/s)
#   u* = -k * c
#
# Because c = S(p) is linear in p, S is folded into W3 on the host:
#   c = h2 @ (W3 @ S) + b3 @ S = h2 @ W3S + b3S
# which shrinks the last matmul 3x and removes all on-device shuffles.
#
# Sharding: pure data parallelism. Batch 8192 is split into 8 shards of
# 1024 rows, one per NeuronCore; the tiny MLP weights are replicated.
#
# Device pipeline per core (batch shard B=1024):
#   - z is cast-DMA'd (SWDGE) to bf16 on load; mm1 contracts over the
#     3072 dim, so z tiles are transposed on-chip through the PE
#     (identity matmul, bf16) into zT panels [128 d x 512 b] consumed as
#     the moving operand of mm1 (bf16, N=512 -> full PE rate + FWL
#     weight loads with 128-wide padded W1 chunks).
#   - Layer-1/2 activations are kept transposed ([feature, batch]); those
#     matmuls run in fp32r (cheap; N=512). b1/b2 are applied as
#     per-partition bias in the tanh activation op.
#   - mm3 produces c in natural layout [128 b x 1024]; b3S is added as a
#     host-prebroadcast [128, 1024] tile; squared row-sums give s;
#     Lambert-W runs via Halley iterations, pipelined in three slices
#     (tiles 0-3 / 4-5 / 6-7) so only the last slice's solve sits on the
#     critical-path tail; c is scaled by -k and streamed out.

import numpy as np
import ml_dtypes
from contextlib import ExitStack

import concourse.bass as bass
import concourse.tile as tile
from concourse import bacc, mybir
from concourse.bass_utils import run_bass_kernel_spmd

F32 = mybir.dt.float32
F32R = mybir.dt.float32r
BF16 = mybir.dt.bfloat16

N_CORES = 8
BATCH = 8192
B = BATCH // N_CORES      # batch rows per core
D = 3072                  # state dim
H = 100                   # hidden
CD = 1024                 # control dim
NCH = D // 128            # 24 contraction chunks for mm1
NBT = B // 128            # 8 batch tiles per core
GROUP = 512               # batch columns per mm1 pass
NG = B // GROUP           # 2 groups per core
TPG = GROUP // 128        # 4 batch tiles per group
MASS = 0.5
N_HALLEY = 5

AF = mybir.ActivationFunctionType
ALU = mybir.AluOpType


def build_kernel():
    nc = bacc.Bacc(None, target_bir_lowering=False, enable_partition_id=False)

    z_d = nc.declare_dram_parameter("z", [B, D], F32, isOutput=False)
    tT_d = nc.declare_dram_parameter("tT", [1, B], F32, isOutput=False)
    w1m_d = nc.declare_dram_parameter("w1m", [128, NCH * 128], BF16, isOutput=False)
    w1e_d = nc.declare_dram_parameter("w1e", [1, 128], BF16, isOutput=False)
    b1c_d = nc.declare_dram_parameter("b1c", [H, 1], F32, isOutput=False)
    w2_d = nc.declare_dram_parameter("w2", [H, H], F32R, isOutput=False)
    b2c_d = nc.declare_dram_parameter("b2c", [H, 1], F32, isOutput=False)
    w3s_d = nc.declare_dram_parameter("w3s", [H, CD], F32R, isOutput=False)
    b3f_d = nc.declare_dram_parameter("b3f", [128, CD], F32, isOutput=False)
    id_d = nc.declare_dram_parameter("ident", [128, 128], BF16, isOutput=False)
    out_d = nc.declare_dram_parameter("out", [B, CD], F32, isOutput=True)

    with ExitStack() as ctx:
        tc = ctx.enter_context(tile.TileContext(nc))

        const = ctx.enter_context(tc.tile_pool(name="const", bufs=1))
        zpool = ctx.enter_context(tc.tile_pool(name="znat", bufs=2 * TPG))
        ztp = ctx.enter_context(tc.tile_pool(name="zt", bufs=3))
        hpool = ctx.enter_context(tc.tile_pool(name="hs", bufs=2))
        cpool = ctx.enter_context(tc.tile_pool(name="call", bufs=1))
        opool = ctx.enter_context(tc.tile_pool(name="outs", bufs=2))
        sqpool = ctx.enter_context(tc.tile_pool(name="sq", bufs=2))
        lwp = ctx.enter_context(tc.tile_pool(name="lw", bufs=1))
        pt_ps = ctx.enter_context(tc.tile_pool(name="ptp", bufs=4, space="PSUM"))
        h1_ps = ctx.enter_context(tc.tile_pool(name="h1p", bufs=1, space="PSUM"))
        h2_ps = ctx.enter_context(tc.tile_pool(name="h2p", bufs=1, space="PSUM"))
        c_ps = ctx.enter_context(tc.tile_pool(name="cp", bufs=2, space="PSUM"))

        # ---- z loads for group 0 go out first (SWDGE, casting f32->bf16);
        # weight DMAs ride HWDGE queues in parallel.
        zn_group = {}
        for g in range(NG):
            zn_group[g] = []

        def load_group(g):
            for q in range(TPG):
                bt = TPG * g + q
                znt = zpool.tile([128, D], BF16, tag="zn", name="zn")
                if bt == 0:
                    # chunked so the first transposes can start early
                    for ck in range(3):
                        cs = ck * (D // 3)
                        nc.gpsimd.dma_start(
                            znt[:, cs:cs + D // 3],
                            z_d[bt * 128:(bt + 1) * 128, cs:cs + D // 3],
                        )
                else:
                    nc.gpsimd.dma_start(znt[:], z_d[bt * 128:(bt + 1) * 128, :])
                zn_group[g].append(znt)

        load_group(0)

        # ---- constants / weights ----
        ident = const.tile([128, 128], BF16, tag="ident")
        nc.sync.dma_start(ident[:], id_d[:])
        w1s = const.tile([128, NCH, 128], BF16, tag="w1s")
        nc.sync.dma_start(w1s[:], w1m_d[:].rearrange("p (c h) -> p c h", c=NCH))
        w1e = const.tile([1, 128], BF16, tag="w1e")
        nc.sync.dma_start(w1e[:], w1e_d[:])
        te = const.tile([1, B], BF16, tag="te")
        nc.gpsimd.dma_start(te[:], tT_d[:])
        b1c = const.tile([H, 1], F32, tag="b1c")
        nc.sync.dma_start(b1c[:], b1c_d[:])
        w2 = const.tile([H, H], F32R, tag="w2")
        nc.sync.dma_start(w2[:], w2_d[:])
        b2c = const.tile([H, 1], F32, tag="b2c")
        nc.sync.dma_start(b2c[:], b2c_d[:])
        w3s = const.tile([H, CD], F32R, tag="w3s")
        nc.sync.dma_start(w3s[:], w3s_d[:])
        b3f = const.tile([128, CD], F32, tag="b3f")
        nc.sync.dma_start(b3f[:], b3f_d[:])

        load_group(1)

        c_all = cpool.tile([128, NBT, CD], F32, tag="c_all")
        s_all = lwp.tile([128, NBT], F32, tag="s_all")

        def lambert_and_store(st, cnt):
            """Solve W for tiles [st, st+cnt) via asymptotic series + two
            log-Newton polish steps, scale c by -k, DMA out."""
            def lt(nm):
                return lwp.tile([128, cnt], F32, tag=f"{nm}{st}", name=f"{nm}{st}")

            for i in range(cnt):
                bt = st + i
                sq = sqpool.tile([128, CD], F32, tag="sq", name="sq")
                nc.scalar.activation(
                    sq[:], c_all[:, bt, :], AF.Square,
                    accum_out=s_all[:, bt:bt + 1],
                )
            sv = s_all[:, st:st + cnt]
            x = lt("lw_x")
            nc.vector.tensor_scalar(x[:], sv, 256.0, 8.0, ALU.mult, ALU.max)
            L1 = lt("lw_L1")
            nc.scalar.activation(L1[:], x[:], AF.Ln)
            L2 = lt("lw_L2")
            nc.scalar.activation(L2[:], L1[:], AF.Ln)
            # w = L1 - L2 + L2/L1 + L2*(L2-2)/(2*L1^2)
            r1 = lt("lw_r1")
            nc.vector.reciprocal(r1[:], L1[:])
            a = lt("lw_a")
            nc.vector.tensor_mul(a[:], L2[:], r1[:])
            w = lt("lw_w")
            nc.vector.tensor_sub(w[:], L1[:], L2[:])
            nc.vector.tensor_add(w[:], w[:], a[:])
            t = lt("lw_t")
            nc.vector.tensor_scalar(t[:], L2[:], -2.0, 0.5, ALU.add, ALU.mult)
            nc.vector.tensor_mul(t[:], t[:], a[:])
            nc.vector.tensor_mul(t[:], t[:], r1[:])
            nc.vector.tensor_add(w[:], w[:], t[:])
            # polish: w -= (ln w + w - ln x) * w / (1 + w)
            g = lt("lw_g")
            wp1 = lt("lw_wp1")
            rcp = lt("lw_rcp")
            for _ in range(2):
                nc.scalar.activation(g[:], w[:], AF.Ln)
                nc.vector.tensor_add(g[:], g[:], w[:])
                nc.vector.tensor_sub(g[:], g[:], L1[:])
                nc.vector.tensor_scalar_add(wp1[:], w[:], 1.0)
                nc.vector.reciprocal(rcp[:], wp1[:])
                nc.vector.tensor_mul(g[:], g[:], w[:])
                nc.vector.tensor_mul(g[:], g[:], rcp[:])
                nc.vector.tensor_sub(w[:], w[:], g[:])
            # kneg = -sqrt(256*w/s)  (0 when s == 0 since then w ~ W(8-guard)*0)
            sg = lt("lw_sg")
            nc.vector.tensor_scalar_max(sg[:], sv, 1e-30)
            nc.vector.reciprocal(rcp[:], sg[:])
            nc.vector.tensor_mul(sg[:], w[:], rcp[:])
            kneg = lt("lw_kneg")
            nc.scalar.activation(kneg[:], sg[:], AF.Sqrt, scale=256.0)
            nc.vector.tensor_scalar_mul(kneg[:], kneg[:], -1.0)
            for i in range(cnt):
                bt = st + i
                ot = opool.tile([128, CD], F32, tag="ot", name="ot")
                nc.scalar.mul(ot[:], c_all[:, bt, :], kneg[:, i:i + 1])
                nc.sync.dma_start(out_d[bt * 128:(bt + 1) * 128, :], ot[:])

        # ---- main loop ----
        # b-tile-major: each batch tile's transposes+mm1 start as soon as
        # ITS z DMA lands; h1p accumulates all four quarters in one PSUM
        # group (the t-column matmul opens the group covering the full
        # bank, so quarter writes accumulate correctly). The tail stages
        # (tanh/mm2/tanh/mm3/c/s) run per-quarter so the last batch tile
        # has a short critical path, and Lambert solves go per tile-pair.
        NJG = NCH // 8  # 3 pt-panels of 8 d-chunks per b-tile

        def tail_group(g, h1p):
            h1s = hpool.tile([H, GROUP], F32R, tag="h1s", name="h1s")
            nc.scalar.activation(h1s[:], h1p[0:H, :], AF.Tanh, bias=b1c[:])
            h2p = h2_ps.tile([H, GROUP], F32, tag="h2p", name="h2p")
            nc.tensor.matmul(h2p[:], w2[:], h1s[:], start=True, stop=True)
            h2s = hpool.tile([H, GROUP], F32R, tag="h2s", name="h2s")
            nc.scalar.activation(h2s[:], h2p[:], AF.Tanh, bias=b2c[:])
            for q in range(TPG):
                bt = TPG * g + q
                for nb in range(2):
                    cp = c_ps.tile([128, 512], F32, tag="cp", name="cp")
                    nc.tensor.matmul(
                        cp[:], h2s[:, q * 128:(q + 1) * 128],
                        w3s[:, nb * 512:(nb + 1) * 512],
                        start=True, stop=True,
                    )
                    # c = cp + b3S  (DVE, PSUM -> SBUF)
                    nc.vector.tensor_add(
                        c_all[:, bt, nb * 512:(nb + 1) * 512],
                        cp[:], b3f[:, nb * 512:(nb + 1) * 512],
                    )

        for g in range(NG):
            zn = zn_group[g]
            work = [(q, jg) for q in range(TPG) for jg in range(NJG)]
            h1p = h1_ps.tile([128, GROUP], F32, tag="h1p", name="h1p")
            # t column opens the accumulation group covering the bank
            nc.tensor.matmul(
                h1p[:], w1e[:], te[:, g * GROUP:(g + 1) * GROUP],
                start=True, stop=False,
            )
            pts = {}

            def emit_transpose(idx):
                q, jg = work[idx]
                pt = pt_ps.tile([128, 1024], BF16, tag="pt", name="pt")
                for u in range(8):
                    j = jg * 8 + u
                    nc.tensor.matmul(
                        pt[:, u * 128:(u + 1) * 128],
                        zn[q][:, j * 128:(j + 1) * 128],
                        ident[:],
                        start=(u == 0), stop=(u == 7),
                        is_transpose=True,
                    )
                pts[idx] = pt

            emit_transpose(0)
            for idx, (q, jg) in enumerate(work):
                if idx + 1 < len(work):
                    emit_transpose(idx + 1)  # keep PE one panel ahead
                zt = ztp.tile([128, 1024], BF16, tag="zt", name="zt")
                nc.vector.tensor_copy(zt[:], pts.pop(idx)[:])
                last = (idx == len(work) - 1)
                for u in range(8):
                    j = jg * 8 + u
                    nc.tensor.matmul(
                        h1p[:, q * 128:(q + 1) * 128],
                        w1s[:, j, :], zt[:, u * 128:(u + 1) * 128],
                        start=False, stop=(last and u == 7),
                    )

            tail_group(g, h1p)
            lambert_and_store(g * TPG, TPG)

    nc.compile()
    return nc


def host_prep(z, t, W1, b1, W2, b2, W3, b3):
    """Host-side weight re-layout + per-core shard maps."""
    f = np.float32
    bf = ml_dtypes.bfloat16
    z = np.asarray(z, f)
    t = np.asarray(t, f)
    W1 = np.asarray(W1, f)
    b1 = np.asarray(b1, f)
    W2 = np.asarray(W2, f)
    b2 = np.asarray(b2, f)
    W3 = np.asarray(W3, f)
    b3 = np.asarray(b3, f)

    # mm1 stationary chunks (bf16, padded to 128 cols for FWL):
    # w1m[p, j*128 + h] = W1[1 + j*128 + p, h]
    w1m = np.zeros((128, NCH, 128), bf)
    w1m[:, :, :H] = W1[1:, :].reshape(NCH, 128, H).transpose(1, 0, 2).astype(bf)
    w1m = np.ascontiguousarray(w1m.reshape(128, NCH * 128))
    w1e = np.zeros((1, 128), bf)
    w1e[0, :H] = W1[0, :].astype(bf)
    b1c = np.ascontiguousarray(b1.reshape(H, 1))
    b2c = np.ascontiguousarray(b2.reshape(H, 1))

    # fold the p -> c map into W3 (and b3)
    W3r = W3.reshape(H, CD // 4, 12)
    W3S = np.empty((H, CD // 4, 4), f)
    W3S[..., 0] = (W3r[..., 6] + W3r[..., 7] + W3r[..., 8]) / MASS
    W3S[..., 1] = W3r[..., 9]
    W3S[..., 2] = W3r[..., 10]
    W3S[..., 3] = W3r[..., 11]
    b3r = b3.reshape(CD // 4, 12)
    b3S = np.empty((CD // 4, 4), f)
    b3S[..., 0] = (b3r[..., 6] + b3r[..., 7] + b3r[..., 8]) / MASS
    b3S[..., 1] = b3r[..., 9]
    b3S[..., 2] = b3r[..., 10]
    b3S[..., 3] = b3r[..., 11]
    w3s = np.ascontiguousarray(W3S.reshape(H, CD))
    b3f = np.ascontiguousarray(np.broadcast_to(b3S.reshape(1, CD), (128, CD)))

    ident = np.eye(128, dtype=bf)

    in_maps = []
    for c in range(N_CORES):
        sl = slice(c * B, (c + 1) * B)
        in_maps.append({
            "z": np.ascontiguousarray(z[sl]),
            "tT": np.ascontiguousarray(t[sl].reshape(1, B)),
            "w1m": w1m,
            "w1e": w1e,
            "b1c": b1c,
            "w2": W2,
            "b2c": b2c,
            "w3s": w3s,
            "b3f": b3f,
            "ident": ident,
        })
    return in_maps


_NC_CACHE = None


def _get_nc():
    global _NC_CACHE
    if _NC_CACHE is None:
        _NC_CACHE = build_kernel()
    return _NC_CACHE


def run(inputs, trace=False):
    """Returns (full_output, BassKernelResults)."""
    nc = _get_nc()
    in_maps = host_prep(**inputs)
    res = run_bass_kernel_spmd(
        nc, in_maps, list(range(N_CORES)), trace=trace,
    )
    out = np.concatenate([r["out"] for r in res.results], axis=0)
    return out.astype(np.float32, copy=False), res


def kernel(**inputs):
    out, _ = run(inputs)
    return out


# revision 13
# speedup vs baseline: 1.1481x; 1.0314x over previous
# Trainium2 Bass kernel for nn_CVXPolicy_MultiQuadcopter.
#
# Math (per sample):
#   x  = concat([t, z])                      (3073,)
#   h1 = tanh(x @ W1 + b1)                   (100,)
#   h2 = tanh(h1 @ W2 + b2)                  (100,)
#   p  = h2 @ W3 + b3                        (3072,)
#   c  = S(p)   (per-agent sparse linear map)   (1024,)
#   s  = ||c||^2 ; w = W(256*s) ; k = sqrt(256*w/s)
#   u* = -k * c
#
# Because c = S(p) is linear in p, S is folded into W3 on the host:
#   c = h2 @ (W3 @ S) + b3 @ S = h2 @ W3S + b3S
# which shrinks the last matmul 3x and removes all on-device shuffles.
#
# Sharding: pure data parallelism. Batch 8192 is split into 8 shards of
# 1024 rows, one per NeuronCore; the tiny MLP weights are replicated.
#
# Device pipeline per core (batch shard B=1024):
#   - z is cast-DMA'd (SWDGE) to bf16 on load; mm1 contracts over the
#     3072 dim, so z tiles are transposed on-chip through the PE
#     (identity matmul, bf16) into zT panels [128 d x 512 b] consumed as
#     the moving operand of mm1 (bf16, N=512 -> full PE rate + FWL
#     weight loads with 128-wide padded W1 chunks).
#   - Layer-1/2 activations are kept transposed ([feature, batch]); those
#     matmuls run in fp32r (cheap; N=512). b1/b2 are applied as
#     per-partition bias in the tanh activation op.
#   - mm3 produces c in natural layout [128 b x 1024]; b3S is added as a
#     host-prebroadcast [128, 1024] tile; squared row-sums give s;
#     Lambert-W runs via Halley iterations, pipelined in three slices
#     (tiles 0-3 / 4-5 / 6-7) so only the last slice's solve sits on the
#     critical-path tail; c is scaled by -k and streamed out.

import numpy as np
import ml_dtypes
from contextlib import ExitStack

import concourse.bass as bass
import concourse.tile as tile
from concourse import bacc, mybir
from concourse.bass_utils import run_bass_kernel_spmd

F32 = mybir.dt.float32
F32R = mybir.dt.float32r
BF16 = mybir.dt.bfloat16

N_CORES = 8
BATCH = 8192
B = BATCH // N_CORES      # batch rows per core
D = 3072                  # state dim
H = 100                   # hidden
CD = 1024                 # control dim
NCH = D // 128            # 24 contraction chunks for mm1
NBT = B // 128            # 8 batch tiles per core
GROUP = 512               # batch columns per mm1 pass
NG = B // GROUP           # 2 groups per core
TPG = GROUP // 128        # 4 batch tiles per group
MASS = 0.5
N_HALLEY = 5

AF = mybir.ActivationFunctionType
ALU = mybir.AluOpType


def build_kernel():
    nc = bacc.Bacc(None, target_bir_lowering=False, enable_partition_id=False)

    z_d = nc.declare_dram_parameter("z", [B, D], F32, isOutput=False)
    tT_d = nc.declare_dram_parameter("tT", [1, B], F32, isOutput=False)
    w1m_d = nc.declare_dram_parameter("w1m", [128, NCH * 128], BF16, isOutput=False)
    w1e_d = nc.declare_dram_parameter("w1e", [1, 128], BF16, isOutput=False)
    b1c_d = nc.declare_dram_parameter("b1c", [H, 1], F32, isOutput=False)
    w2_d = nc.declare_dram_parameter("w2", [H, H], F32R, isOutput=False)
    b2c_d = nc.declare_dram_parameter("b2c", [H, 1], F32, isOutput=False)
    w3s_d = nc.declare_dram_parameter("w3s", [H, CD], F32R, isOutput=False)
    b3f_d = nc.declare_dram_parameter("b3f", [128, CD], F32, isOutput=False)
    id_d = nc.declare_dram_parameter("ident", [128, 128], BF16, isOutput=False)
    out_d = nc.declare_dram_parameter("out", [B, CD], F32, isOutput=True)

    with ExitStack() as ctx:
        tc = ctx.enter_context(tile.TileContext(nc))

        const = ctx.enter_context(tc.tile_pool(name="const", bufs=1))
        zpool = ctx.enter_context(tc.tile_pool(name="znat", bufs=2 * TPG))
        ztp = ctx.enter_context(tc.tile_pool(name="zt", bufs=3))
        hpool = ctx.enter_context(tc.tile_pool(name="hs", bufs=2))
        cpool = ctx.enter_context(tc.tile_pool(name="call", bufs=1))
        opool = ctx.enter_context(tc.tile_pool(name="outs", bufs=2))
        sqpool = ctx.enter_context(tc.tile_pool(name="sq", bufs=2))
        lwp = ctx.enter_context(tc.tile_pool(name="lw", bufs=1))
        pt_ps = ctx.enter_context(tc.tile_pool(name="ptp", bufs=4, space="PSUM"))
        h1_ps = ctx.enter_context(tc.tile_pool(name="h1p", bufs=1, space="PSUM"))
        h2_ps = ctx.enter_context(tc.tile_pool(name="h2p", bufs=1, space="PSUM"))
        c_ps = ctx.enter_context(tc.tile_pool(name="cp", bufs=2, space="PSUM"))

        # ---- z loads for group 0 go out first (SWDGE, casting f32->bf16);
        # weight DMAs ride HWDGE queues in parallel.
        zn_group = {}
        for g in range(NG):
            zn_group[g] = []

        def load_group(g):
            for q in range(TPG):
                bt = TPG * g + q
                znt = zpool.tile([128, D], BF16, tag="zn", name="zn")
                if bt <= 1:
                    # chunked so the first transposes can start early
                    for ck in range(3):
                        cs = ck * (D // 3)
                        nc.gpsimd.dma_start(
                            znt[:, cs:cs + D // 3],
                            z_d[bt * 128:(bt + 1) * 128, cs:cs + D // 3],
                        )
                else:
                    nc.gpsimd.dma_start(znt[:], z_d[bt * 128:(bt + 1) * 128, :])
                zn_group[g].append(znt)

        load_group(0)

        # ---- constants / weights ----
        ident = const.tile([128, 128], BF16, tag="ident")
        nc.sync.dma_start(ident[:], id_d[:])
        w1s = const.tile([128, NCH, 128], BF16, tag="w1s")
        nc.sync.dma_start(w1s[:], w1m_d[:].rearrange("p (c h) -> p c h", c=NCH))
        w1e = const.tile([1, 128], BF16, tag="w1e")
        nc.sync.dma_start(w1e[:], w1e_d[:])
        te = const.tile([1, B], BF16, tag="te")
        nc.gpsimd.dma_start(te[:], tT_d[:])
        b1c = const.tile([H, 1], F32, tag="b1c")
        nc.sync.dma_start(b1c[:], b1c_d[:])
        w2 = const.tile([H, H], F32R, tag="w2")
        nc.sync.dma_start(w2[:], w2_d[:])
        b2c = const.tile([H, 1], F32, tag="b2c")
        nc.sync.dma_start(b2c[:], b2c_d[:])
        w3s = const.tile([H, CD], F32R, tag="w3s")
        nc.sync.dma_start(w3s[:], w3s_d[:])
        b3f = const.tile([128, CD], F32, tag="b3f")
        nc.sync.dma_start(b3f[:], b3f_d[:])

        load_group(1)

        c_all = cpool.tile([128, NBT, CD], F32, tag="c_all")
        s_all = lwp.tile([128, NBT], F32, tag="s_all")

        def lambert_and_store(st, cnt):
            """Solve W for tiles [st, st+cnt) via asymptotic series + two
            log-Newton polish steps, scale c by -k, DMA out."""
            def lt(nm):
                return lwp.tile([128, cnt], F32, tag=f"{nm}{st}", name=f"{nm}{st}")

            sv = s_all[:, st:st + cnt]
            x = lt("lw_x")
            nc.vector.tensor_scalar(x[:], sv, 256.0, 8.0, ALU.mult, ALU.max)
            L1 = lt("lw_L1")
            nc.scalar.activation(L1[:], x[:], AF.Ln)
            L2 = lt("lw_L2")
            nc.scalar.activation(L2[:], L1[:], AF.Ln)
            # w = L1 - L2 + L2/L1 + L2*(L2-2)/(2*L1^2)
            r1 = lt("lw_r1")
            nc.vector.reciprocal(r1[:], L1[:])
            a = lt("lw_a")
            nc.vector.tensor_mul(a[:], L2[:], r1[:])
            w = lt("lw_w")
            nc.vector.tensor_sub(w[:], L1[:], L2[:])
            nc.vector.tensor_add(w[:], w[:], a[:])
            t = lt("lw_t")
            nc.vector.tensor_scalar(t[:], L2[:], -2.0, 0.5, ALU.add, ALU.mult)
            nc.vector.tensor_mul(t[:], t[:], a[:])
            nc.vector.tensor_mul(t[:], t[:], r1[:])
            nc.vector.tensor_add(w[:], w[:], t[:])
            # polish: w -= (ln w + w - ln x) * w / (1 + w)
            g = lt("lw_g")
            wp1 = lt("lw_wp1")
            rcp = lt("lw_rcp")
            for _ in range(1):
                nc.scalar.activation(g[:], w[:], AF.Ln)
                nc.vector.tensor_add(g[:], g[:], w[:])
                nc.vector.tensor_sub(g[:], g[:], L1[:])
                nc.vector.tensor_scalar_add(wp1[:], w[:], 1.0)
                nc.vector.reciprocal(rcp[:], wp1[:])
                nc.vector.tensor_mul(g[:], g[:], w[:])
                nc.vector.tensor_mul(g[:], g[:], rcp[:])
                nc.vector.tensor_sub(w[:], w[:], g[:])
            # kneg = -sqrt(256*w/s)  (0 when s == 0 since then w ~ W(8-guard)*0)
            sg = lt("lw_sg")
            nc.vector.tensor_scalar_max(sg[:], sv, 1e-30)
            nc.vector.reciprocal(rcp[:], sg[:])
            nc.vector.tensor_mul(sg[:], w[:], rcp[:])
            kneg = lt("lw_kneg")
            nc.scalar.activation(kneg[:], sg[:], AF.Sqrt, scale=256.0)
            nc.vector.tensor_scalar_mul(kneg[:], kneg[:], -1.0)
            for i in range(cnt):
                bt = st + i
                ot = opool.tile([128, CD], F32, tag="ot", name="ot")
                nc.vector.tensor_scalar_mul(ot[:], c_all[:, bt, :], kneg[:, i:i + 1])
                nc.sync.dma_start(out_d[bt * 128:(bt + 1) * 128, :], ot[:])

        # ---- main loop ----
        # b-tile-major: each batch tile's transposes+mm1 start as soon as
        # ITS z DMA lands; h1p accumulates all four quarters in one PSUM
        # group (the t-column matmul opens the group covering the full
        # bank, so quarter writes accumulate correctly). The tail stages
        # (tanh/mm2/tanh/mm3/c/s) run per-quarter so the last batch tile
        # has a short critical path, and Lambert solves go per tile-pair.
        NJG = NCH // 8  # 3 pt-panels of 8 d-chunks per b-tile

        def tail_group(g, h1p):
            h1s = hpool.tile([H, GROUP], F32R, tag="h1s", name="h1s")
            nc.scalar.activation(h1s[:], h1p[0:H, :], AF.Tanh, bias=b1c[:])
            h2p = h2_ps.tile([H, GROUP], F32, tag="h2p", name="h2p")
            nc.tensor.matmul(h2p[:], w2[:], h1s[:], start=True, stop=True)
            h2s = hpool.tile([H, GROUP], F32R, tag="h2s", name="h2s")
            nc.scalar.activation(h2s[:], h2p[:], AF.Tanh, bias=b2c[:])
            for q in range(TPG):
                bt = TPG * g + q
                for nb in range(2):
                    cp = c_ps.tile([128, 512], F32, tag="cp", name="cp")
                    nc.tensor.matmul(
                        cp[:], h2s[:, q * 128:(q + 1) * 128],
                        w3s[:, nb * 512:(nb + 1) * 512],
                        start=True, stop=True,
                    )
                    # c = cp + b3S  (DVE, PSUM -> SBUF)
                    nc.vector.tensor_add(
                        c_all[:, bt, nb * 512:(nb + 1) * 512],
                        cp[:], b3f[:, nb * 512:(nb + 1) * 512],
                    )
                sq = sqpool.tile([128, CD], F32, tag="sq", name="sq")
                nc.scalar.activation(
                    sq[:], c_all[:, bt, :], AF.Square,
                    accum_out=s_all[:, bt:bt + 1],
                )

        for g in range(NG):
            zn = zn_group[g]
            work = [(q, jg) for q in range(TPG) for jg in range(NJG)]
            h1p = h1_ps.tile([128, GROUP], F32, tag="h1p", name="h1p")
            # t column opens the accumulation group covering the bank
            nc.tensor.matmul(
                h1p[:], w1e[:], te[:, g * GROUP:(g + 1) * GROUP],
                start=True, stop=False,
            )
            pts = {}

            def emit_transpose(idx):
                q, jg = work[idx]
                pt = pt_ps.tile([128, 1024], BF16, tag="pt", name="pt")
                for u in range(8):
                    j = jg * 8 + u
                    nc.tensor.matmul(
                        pt[:, u * 128:(u + 1) * 128],
                        zn[q][:, j * 128:(j + 1) * 128],
                        ident[:],
                        start=(u == 0), stop=(u == 7),
                        is_transpose=True,
                    )
                pts[idx] = pt

            emit_transpose(0)
            for idx, (q, jg) in enumerate(work):
                if idx + 1 < len(work):
                    emit_transpose(idx + 1)  # keep PE one panel ahead
                zt = ztp.tile([128, 1024], BF16, tag="zt", name="zt")
                nc.vector.tensor_copy(zt[:], pts.pop(idx)[:])
                last = (idx == len(work) - 1)
                for u in range(8):
                    j = jg * 8 + u
                    nc.tensor.matmul(
                        h1p[:, q * 128:(q + 1) * 128],
                        w1s[:, j, :], zt[:, u * 128:(u + 1) * 128],
                        start=False, stop=(last and u == 7),
                    )

            tail_group(g, h1p)
            lambert_and_store(g * TPG, TPG)

    nc.compile()
    return nc


def host_prep(z, t, W1, b1, W2, b2, W3, b3):
    """Host-side weight re-layout + per-core shard maps."""
    f = np.float32
    bf = ml_dtypes.bfloat16
    z = np.asarray(z, f)
    t = np.asarray(t, f)
    W1 = np.asarray(W1, f)
    b1 = np.asarray(b1, f)
    W2 = np.asarray(W2, f)
    b2 = np.asarray(b2, f)
    W3 = np.asarray(W3, f)
    b3 = np.asarray(b3, f)

    # mm1 stationary chunks (bf16, padded to 128 cols for FWL):
    # w1m[p, j*128 + h] = W1[1 + j*128 + p, h]
    w1m = np.zeros((128, NCH, 128), bf)
    w1m[:, :, :H] = W1[1:, :].reshape(NCH, 128, H).transpose(1, 0, 2).astype(bf)
    w1m = np.ascontiguousarray(w1m.reshape(128, NCH * 128))
    w1e = np.zeros((1, 128), bf)
    w1e[0, :H] = W1[0, :].astype(bf)
    b1c = np.ascontiguousarray(b1.reshape(H, 1))
    b2c = np.ascontiguousarray(b2.reshape(H, 1))

    # fold the p -> c map into W3 (and b3)
    W3r = W3.reshape(H, CD // 4, 12)
    W3S = np.empty((H, CD // 4, 4), f)
    W3S[..., 0] = (W3r[..., 6] + W3r[..., 7] + W3r[..., 8]) / MASS
    W3S[..., 1] = W3r[..., 9]
    W3S[..., 2] = W3r[..., 10]
    W3S[..., 3] = W3r[..., 11]
    b3r = b3.reshape(CD // 4, 12)
    b3S = np.empty((CD // 4, 4), f)
    b3S[..., 0] = (b3r[..., 6] + b3r[..., 7] + b3r[..., 8]) / MASS
    b3S[..., 1] = b3r[..., 9]
    b3S[..., 2] = b3r[..., 10]
    b3S[..., 3] = b3r[..., 11]
    w3s = np.ascontiguousarray(W3S.reshape(H, CD))
    b3f = np.ascontiguousarray(np.broadcast_to(b3S.reshape(1, CD), (128, CD)))

    ident = np.eye(128, dtype=bf)

    in_maps = []
    for c in range(N_CORES):
        sl = slice(c * B, (c + 1) * B)
        in_maps.append({
            "z": np.ascontiguousarray(z[sl]),
            "tT": np.ascontiguousarray(t[sl].reshape(1, B)),
            "w1m": w1m,
            "w1e": w1e,
            "b1c": b1c,
            "w2": W2,
            "b2c": b2c,
            "w3s": w3s,
            "b3f": b3f,
            "ident": ident,
        })
    return in_maps


_NC_CACHE = None


def _get_nc():
    global _NC_CACHE
    if _NC_CACHE is None:
        _NC_CACHE = build_kernel()
    return _NC_CACHE


def run(inputs, trace=False):
    """Returns (full_output, BassKernelResults)."""
    nc = _get_nc()
    in_maps = host_prep(**inputs)
    res = run_bass_kernel_spmd(
        nc, in_maps, list(range(N_CORES)), trace=trace,
    )
    out = np.concatenate([r["out"] for r in res.results], axis=0)
    return out.astype(np.float32, copy=False), res


def kernel(**inputs):
    out, _ = run(inputs)
    return out


# revision 15
# speedup vs baseline: 1.2039x; 1.0486x over previous
# Trainium2 Bass kernel for nn_CVXPolicy_MultiQuadcopter.
#
# Math (per sample):
#   x  = concat([t, z])                      (3073,)
#   h1 = tanh(x @ W1 + b1)                   (100,)
#   h2 = tanh(h1 @ W2 + b2)                  (100,)
#   p  = h2 @ W3 + b3                        (3072,)
#   c  = S(p)   (per-agent sparse linear map)   (1024,)
#   s  = ||c||^2 ; w = W(256*s) ; k = sqrt(256*w/s)
#   u* = -k * c
#
# Because c = S(p) is linear in p, S is folded into W3 on the host:
#   c = h2 @ (W3 @ S) + b3 @ S = h2 @ W3S + b3S
# which shrinks the last matmul 3x and removes all on-device shuffles.
#
# Sharding: pure data parallelism. Batch 8192 is split into 8 shards of
# 1024 rows, one per NeuronCore; the tiny MLP weights are replicated.
#
# Device pipeline per core (batch shard B=1024):
#   - z is cast-DMA'd (SWDGE) to bf16 on load; mm1 contracts over the
#     3072 dim, so z tiles are transposed on-chip through the PE
#     (identity matmul, bf16, batched 8 chunks per PSUM bank) and copied
#     to SBUF by the DVE, then consumed as the moving operand of mm1
#     (bf16, FWL weight loads via 128-wide padded W1 chunks). The
#     pipeline is batch-tile-major so compute starts as soon as the
#     first z tile lands.
#   - Layer-1/2 activations are kept transposed ([feature, batch]); those
#     matmuls run in fp32r. b1/b2 are applied as per-partition bias in
#     the tanh activation; tails run per half-group (256 batch) to keep
#     the last tile's critical path short.
#   - mm3 produces c in natural layout [128 b x 1024]; b3S is added as a
#     host-prebroadcast [128, 1024] tile; squared row-sums give s
#     (fused activation accumulate); Lambert-W is solved by an
#     asymptotic series + one log-Newton polish; c is scaled by -k and
#     streamed out. The ACT engine function-table rotation is kept
#     minimal (table loads cost ~1.3us each).

import numpy as np
import ml_dtypes
from contextlib import ExitStack

import concourse.bass as bass
import concourse.tile as tile
from concourse import bacc, mybir
from concourse.bass_utils import run_bass_kernel_spmd

F32 = mybir.dt.float32
F32R = mybir.dt.float32r
BF16 = mybir.dt.bfloat16

N_CORES = 8
BATCH = 8192
B = BATCH // N_CORES      # batch rows per core
D = 3072                  # state dim
H = 100                   # hidden
CD = 1024                 # control dim
NCH = D // 128            # 24 contraction chunks for mm1
NBT = B // 128            # 8 batch tiles per core
GROUP = 512               # batch columns per outer pass
NG = B // GROUP           # 2 groups per core
TPG = GROUP // 128        # 4 batch tiles per group
NJG = NCH // 8            # 3 transpose panels (of 8 chunks) per b-tile
MASS = 0.5

AF = mybir.ActivationFunctionType
ALU = mybir.AluOpType


def build_kernel():
    nc = bacc.Bacc(None, target_bir_lowering=False, enable_partition_id=False)

    z_d = nc.declare_dram_parameter("z", [B, D], F32, isOutput=False)
    tT_d = nc.declare_dram_parameter("tT", [1, B], F32, isOutput=False)
    w1m_d = nc.declare_dram_parameter("w1m", [128, NCH * 128], BF16, isOutput=False)
    w1e_d = nc.declare_dram_parameter("w1e", [1, 128], BF16, isOutput=False)
    b1c_d = nc.declare_dram_parameter("b1c", [H, 1], F32, isOutput=False)
    w2_d = nc.declare_dram_parameter("w2", [H, H], F32R, isOutput=False)
    b2c_d = nc.declare_dram_parameter("b2c", [H, 1], F32, isOutput=False)
    w3s_d = nc.declare_dram_parameter("w3s", [H, CD], F32R, isOutput=False)
    b3f_d = nc.declare_dram_parameter("b3f", [128, CD], F32, isOutput=False)
    id_d = nc.declare_dram_parameter("ident", [128, 128], BF16, isOutput=False)
    out_d = nc.declare_dram_parameter("out", [B, CD], F32, isOutput=True)

    with ExitStack() as ctx:
        tc = ctx.enter_context(tile.TileContext(nc))

        const = ctx.enter_context(tc.tile_pool(name="const", bufs=1))
        zpool = ctx.enter_context(tc.tile_pool(name="znat", bufs=2 * TPG))
        ztp = ctx.enter_context(tc.tile_pool(name="zt", bufs=3))
        hpool = ctx.enter_context(tc.tile_pool(name="hs", bufs=2))
        cpool = ctx.enter_context(tc.tile_pool(name="call", bufs=1))
        opool = ctx.enter_context(tc.tile_pool(name="outs", bufs=2))
        sqpool = ctx.enter_context(tc.tile_pool(name="sq", bufs=2))
        lwp = ctx.enter_context(tc.tile_pool(name="lw", bufs=1))
        pt_ps = ctx.enter_context(tc.tile_pool(name="ptp", bufs=2, space="PSUM"))
        h1_ps = ctx.enter_context(tc.tile_pool(name="h1p", bufs=2, space="PSUM"))
        h2_ps = ctx.enter_context(tc.tile_pool(name="h2p", bufs=2, space="PSUM"))
        c_ps = ctx.enter_context(tc.tile_pool(name="cp", bufs=2, space="PSUM"))

        # ---- z loads for group 0 go out first (SWDGE, casting f32->bf16);
        # weight DMAs ride HWDGE queues in parallel. The first two tiles
        # are column-chunked so the PE can start transposing early.
        zn_group = {g: [] for g in range(NG)}

        def load_group(g):
            for q in range(TPG):
                bt = TPG * g + q
                znt = zpool.tile([128, D], BF16, tag="zn", name="zn")
                if bt <= 1:
                    for ck in range(3):
                        cs = ck * (D // 3)
                        nc.gpsimd.dma_start(
                            znt[:, cs:cs + D // 3],
                            z_d[bt * 128:(bt + 1) * 128, cs:cs + D // 3],
                        )
                else:
                    nc.gpsimd.dma_start(znt[:], z_d[bt * 128:(bt + 1) * 128, :])
                zn_group[g].append(znt)

        load_group(0)

        # ---- constants / weights ----
        ident = const.tile([128, 128], BF16, tag="ident")
        nc.sync.dma_start(ident[:], id_d[:])
        w1s = const.tile([128, NCH, 128], BF16, tag="w1s")
        nc.sync.dma_start(w1s[:], w1m_d[:].rearrange("p (c h) -> p c h", c=NCH))
        w1e = const.tile([1, 128], BF16, tag="w1e")
        nc.sync.dma_start(w1e[:], w1e_d[:])
        te = const.tile([1, B], BF16, tag="te")
        nc.gpsimd.dma_start(te[:], tT_d[:])
        b1c = const.tile([H, 1], F32, tag="b1c")
        nc.sync.dma_start(b1c[:], b1c_d[:])
        w2 = const.tile([H, H], F32R, tag="w2")
        nc.sync.dma_start(w2[:], w2_d[:])
        b2c = const.tile([H, 1], F32, tag="b2c")
        nc.sync.dma_start(b2c[:], b2c_d[:])
        w3s = const.tile([H, CD], F32R, tag="w3s")
        nc.sync.dma_start(w3s[:], w3s_d[:])
        b3f = const.tile([128, CD], F32, tag="b3f")
        nc.sync.dma_start(b3f[:], b3f_d[:])

        load_group(1)

        c_all = cpool.tile([128, NBT, CD], F32, tag="c_all")
        s_all = lwp.tile([128, NBT], F32, tag="s_all")

        def lambert_and_store(st, cnt):
            """Solve W for tiles [st, st+cnt) via asymptotic series + one
            log-Newton polish step, scale c by -k, DMA out."""
            def lt(nm):
                return lwp.tile([128, cnt], F32, tag=f"{nm}{st}", name=f"{nm}{st}")

            sv = s_all[:, st:st + cnt]
            x = lt("lw_x")
            nc.vector.tensor_scalar(x[:], sv, 256.0, 8.0, ALU.mult, ALU.max)
            L1 = lt("lw_L1")
            nc.scalar.activation(L1[:], x[:], AF.Ln)
            L2 = lt("lw_L2")
            nc.scalar.activation(L2[:], L1[:], AF.Ln)
            # w = L1 - L2 + L2/L1 + L2*(L2-2)/(2*L1^2)
            r1 = lt("lw_r1")
            nc.vector.reciprocal(r1[:], L1[:])
            a = lt("lw_a")
            nc.vector.tensor_mul(a[:], L2[:], r1[:])
            w = lt("lw_w")
            nc.vector.tensor_sub(w[:], L1[:], L2[:])
            nc.vector.tensor_add(w[:], w[:], a[:])
            t = lt("lw_t")
            nc.vector.tensor_scalar(t[:], L2[:], -2.0, 0.5, ALU.add, ALU.mult)
            nc.vector.tensor_mul(t[:], t[:], a[:])
            nc.vector.tensor_mul(t[:], t[:], r1[:])
            nc.vector.tensor_add(w[:], w[:], t[:])
            # polish: w -= (ln w + w - ln x) * w / (1 + w)
            g = lt("lw_g")
            wp1 = lt("lw_wp1")
            rcp = lt("lw_rcp")
            nc.scalar.activation(g[:], w[:], AF.Ln)
            nc.vector.tensor_add(g[:], g[:], w[:])
            nc.vector.tensor_sub(g[:], g[:], L1[:])
            nc.vector.tensor_scalar_add(wp1[:], w[:], 1.0)
            nc.vector.reciprocal(rcp[:], wp1[:])
            nc.vector.tensor_mul(g[:], g[:], w[:])
            nc.vector.tensor_mul(g[:], g[:], rcp[:])
            nc.vector.tensor_sub(w[:], w[:], g[:])
            # kneg = -sqrt(256*w/s)  (0 when s == 0: w*rcp(s-guard) ~ 0)
            sg = lt("lw_sg")
            nc.vector.tensor_scalar_max(sg[:], sv, 1e-30)
            nc.vector.reciprocal(rcp[:], sg[:])
            nc.vector.tensor_mul(sg[:], w[:], rcp[:])
            kneg = lt("lw_kneg")
            nc.scalar.activation(kneg[:], sg[:], AF.Sqrt, scale=256.0)
            nc.vector.tensor_scalar_mul(kneg[:], kneg[:], -1.0)
            for i in range(cnt):
                bt = st + i
                ot = opool.tile([128, CD], F32, tag="ot", name="ot")
                nc.vector.tensor_scalar_mul(
                    ot[:], c_all[:, bt, :], kneg[:, i:i + 1]
                )
                nc.sync.dma_start(out_d[bt * 128:(bt + 1) * 128, :], ot[:])

        def tail_half(g, hf, h1p):
            # process quarters [2*hf, 2*hf+1] of group g (h1p is [128, 256])
            h1s = hpool.tile([H, 256], F32R, tag="h1s", name="h1s")
            nc.scalar.activation(h1s[:], h1p[0:H, :], AF.Tanh, bias=b1c[:])
            h2p = h2_ps.tile([H, 256], F32, tag="h2p", name="h2p")
            nc.tensor.matmul(h2p[:], w2[:], h1s[:], start=True, stop=True)
            h2s = hpool.tile([H, 256], F32R, tag="h2s", name="h2s")
            nc.scalar.activation(h2s[:], h2p[:], AF.Tanh, bias=b2c[:])
            for qq in range(2):
                bt = TPG * g + 2 * hf + qq
                for nb in range(2):
                    cp = c_ps.tile([128, 512], F32, tag="cp", name="cp")
                    nc.tensor.matmul(
                        cp[:], h2s[:, qq * 128:(qq + 1) * 128],
                        w3s[:, nb * 512:(nb + 1) * 512],
                        start=True, stop=True,
                    )
                    # c = cp + b3S  (DVE, PSUM -> SBUF)
                    nc.vector.tensor_add(
                        c_all[:, bt, nb * 512:(nb + 1) * 512],
                        cp[:], b3f[:, nb * 512:(nb + 1) * 512],
                    )
                sq = sqpool.tile([128, CD], F32, tag="sq", name="sq")
                nc.scalar.activation(
                    sq[:], c_all[:, bt, :], AF.Square,
                    accum_out=s_all[:, bt:bt + 1],
                )

        # ---- main loop: batch-tile-major z pipeline, half-group tails ----
        for g in range(NG):
            zn = zn_group[g]
            work = [(q, jg) for q in range(TPG) for jg in range(NJG)]
            h1ps = {}
            for hf in range(2):
                h1ps[hf] = h1_ps.tile([128, 256], F32, tag="h1p", name="h1p")
                # t column opens the half's accumulation group
                cst = (g * TPG + 2 * hf) * 128
                nc.tensor.matmul(
                    h1ps[hf][:], w1e[:], te[:, cst:cst + 256],
                    start=True, stop=False,
                )
            pts = {}

            def emit_transpose(idx):
                q, jg = work[idx]
                pt = pt_ps.tile([128, 1024], BF16, tag="pt", name="pt")
                for u in range(8):
                    j = jg * 8 + u
                    nc.tensor.matmul(
                        pt[:, u * 128:(u + 1) * 128],
                        zn[q][:, j * 128:(j + 1) * 128],
                        ident[:],
                        start=(u == 0), stop=(u == 7),
                        is_transpose=True,
                    )
                pts[idx] = pt

            emit_transpose(0)
            for idx, (q, jg) in enumerate(work):
                if idx + 1 < len(work):
                    emit_transpose(idx + 1)  # keep PE one panel ahead
                zt = ztp.tile([128, 1024], BF16, tag="zt", name="zt")
                nc.vector.tensor_copy(zt[:], pts.pop(idx)[:])
                hf, qq = q // 2, q % 2
                lastq = (qq == 1 and jg == NJG - 1)
                for u in range(8):
                    j = jg * 8 + u
                    nc.tensor.matmul(
                        h1ps[hf][:, qq * 128:(qq + 1) * 128],
                        w1s[:, j, :], zt[:, u * 128:(u + 1) * 128],
                        start=False, stop=(lastq and u == 7),
                    )
                if lastq:
                    tail_half(g, hf, h1ps.pop(hf))
                    if hf == 1:
                        lambert_and_store(g * TPG, TPG)

    nc.compile()
    return nc


def host_prep(z, t, W1, b1, W2, b2, W3, b3):
    """Host-side weight re-layout + per-core shard maps."""
    f = np.float32
    bf = ml_dtypes.bfloat16
    z = np.asarray(z, f)
    t = np.asarray(t, f)
    W1 = np.asarray(W1, f)
    b1 = np.asarray(b1, f)
    W2 = np.asarray(W2, f)
    b2 = np.asarray(b2, f)
    W3 = np.asarray(W3, f)
    b3 = np.asarray(b3, f)

    # mm1 stationary chunks (bf16, padded to 128 cols for FWL):
    # w1m[p, j*128 + h] = W1[1 + j*128 + p, h]
    w1m = np.zeros((128, NCH, 128), bf)
    w1m[:, :, :H] = W1[1:, :].reshape(NCH, 128, H).transpose(1, 0, 2).astype(bf)
    w1m = np.ascontiguousarray(w1m.reshape(128, NCH * 128))
    w1e = np.zeros((1, 128), bf)
    w1e[0, :H] = W1[0, :].astype(bf)
    b1c = np.ascontiguousarray(b1.reshape(H, 1))
    b2c = np.ascontiguousarray(b2.reshape(H, 1))

    # fold the p -> c map into W3 (and b3)
    W3r = W3.reshape(H, CD // 4, 12)
    W3S = np.empty((H, CD // 4, 4), f)
    W3S[..., 0] = (W3r[..., 6] + W3r[..., 7] + W3r[..., 8]) / MASS
    W3S[..., 1] = W3r[..., 9]
    W3S[..., 2] = W3r[..., 10]
    W3S[..., 3] = W3r[..., 11]
    b3r = b3.reshape(CD // 4, 12)
    b3S = np.empty((CD // 4, 4), f)
    b3S[..., 0] = (b3r[..., 6] + b3r[..., 7] + b3r[..., 8]) / MASS
    b3S[..., 1] = b3r[..., 9]
    b3S[..., 2] = b3r[..., 10]
    b3S[..., 3] = b3r[..., 11]
    w3s = np.ascontiguousarray(W3S.reshape(H, CD))
    b3f = np.ascontiguousarray(np.broadcast_to(b3S.reshape(1, CD), (128, CD)))

    ident = np.eye(128, dtype=bf)

    in_maps = []
    for c in range(N_CORES):
        sl = slice(c * B, (c + 1) * B)
        in_maps.append({
            "z": np.ascontiguousarray(z[sl]),
            "tT": np.ascontiguousarray(t[sl].reshape(1, B)),
            "w1m": w1m,
            "w1e": w1e,
            "b1c": b1c,
            "w2": W2,
            "b2c": b2c,
            "w3s": w3s,
            "b3f": b3f,
            "ident": ident,
        })
    return in_maps


_NC_CACHE = None


def _get_nc():
    global _NC_CACHE
    if _NC_CACHE is None:
        _NC_CACHE = build_kernel()
    return _NC_CACHE


def run(inputs, trace=False):
    """Returns (full_output, BassKernelResults)."""
    nc = _get_nc()
    in_maps = host_prep(**inputs)
    res = run_bass_kernel_spmd(
        nc, in_maps, list(range(N_CORES)), trace=trace,
    )
    out = np.concatenate([r["out"] for r in res.results], axis=0)
    return out.astype(np.float32, copy=False), res


def kernel(**inputs):
    out, _ = run(inputs)
    return out


# revision 20
# speedup vs baseline: 1.3179x; 1.0948x over previous
# Trainium2 Bass kernel for nn_CVXPolicy_MultiQuadcopter.
#
# Math (per sample):
#   x  = concat([t, z])                      (3073,)
#   h1 = tanh(x @ W1 + b1)                   (100,)
#   h2 = tanh(h1 @ W2 + b2)                  (100,)
#   p  = h2 @ W3 + b3                        (3072,)
#   c  = S(p)   (per-agent sparse linear map)   (1024,)
#   s  = ||c||^2 ; w = W(256*s) ; k = sqrt(256*w/s)
#   u* = -k * c
#
# Because c = S(p) is linear in p, S is folded into W3 on the host:
#   c = h2 @ (W3 @ S) + b3 @ S = h2 @ W3S + b3S
# which shrinks the last matmul 3x and removes all on-device shuffles.
#
# Sharding: pure data parallelism. Batch 8192 is split into 8 shards of
# 1024 rows, one per NeuronCore; the tiny MLP weights are replicated.
#
# Device pipeline per core (batch shard B=1024):
#   - z is cast-DMA'd (SWDGE) to bf16 on load; mm1 contracts over the
#     3072 dim, so z tiles are transposed on-chip through the PE
#     (identity matmul, bf16, batched 8 chunks per PSUM bank) and copied
#     to SBUF by the DVE, then consumed as the moving operand of mm1
#     (bf16, FWL weight loads via 128-wide padded W1 chunks). The
#     pipeline is batch-tile-major so compute starts as soon as the
#     first z tile lands.
#   - Layer-1/2 activations are kept transposed ([feature, batch]); those
#     matmuls run in fp32r. b1/b2 are applied as per-partition bias in
#     the tanh activation; tails run per half-group (256 batch) to keep
#     the last tile's critical path short.
#   - mm3 produces c in natural layout [128 b x 1024]; b3S is added as a
#     host-prebroadcast [128, 1024] tile; squared row-sums give s
#     (fused activation accumulate); Lambert-W is solved by an
#     asymptotic series + one log-Newton polish; c is scaled by -k and
#     streamed out. The ACT engine function-table rotation is kept
#     minimal (table loads cost ~1.3us each).

import numpy as np
import ml_dtypes
from contextlib import ExitStack

import concourse.bass as bass
import concourse.tile as tile
from concourse import bacc, mybir
from concourse.bass_utils import run_bass_kernel_spmd

F32 = mybir.dt.float32
F32R = mybir.dt.float32r
BF16 = mybir.dt.bfloat16

N_CORES = 8
BATCH = 8192
B = BATCH // N_CORES      # batch rows per core
D = 3072                  # state dim
H = 100                   # hidden
CD = 1024                 # control dim
NCH = D // 128            # 24 contraction chunks for mm1
NBT = B // 128            # 8 batch tiles per core
GROUP = 512               # batch columns per outer pass
NG = B // GROUP           # 2 groups per core
TPG = GROUP // 128        # 4 batch tiles per group
NJG = NCH // 8            # 3 transpose panels (of 8 chunks) per b-tile
MASS = 0.5

AF = mybir.ActivationFunctionType
ALU = mybir.AluOpType


def build_kernel():
    nc = bacc.Bacc(None, target_bir_lowering=False, enable_partition_id=False)

    z_d = nc.declare_dram_parameter("z", [B, D], F32, isOutput=False)
    tT_d = nc.declare_dram_parameter("tT", [1, B], F32, isOutput=False)
    w1m_d = nc.declare_dram_parameter("w1m", [128, NCH * 128], BF16, isOutput=False)
    w1e_d = nc.declare_dram_parameter("w1e", [1, 128], BF16, isOutput=False)
    b1c_d = nc.declare_dram_parameter("b1c", [H, 1], F32, isOutput=False)
    w2_d = nc.declare_dram_parameter("w2", [H, H], F32R, isOutput=False)
    b2c_d = nc.declare_dram_parameter("b2c", [H, 1], F32, isOutput=False)
    w3s_d = nc.declare_dram_parameter("w3s", [H, CD], F32R, isOutput=False)
    b3f_d = nc.declare_dram_parameter("b3f", [128, CD], F32, isOutput=False)
    id_d = nc.declare_dram_parameter("ident", [128, 128], BF16, isOutput=False)
    out_d = nc.declare_dram_parameter("out", [B, CD], F32, isOutput=True)

    with ExitStack() as ctx:
        tc = ctx.enter_context(tile.TileContext(nc))

        const = ctx.enter_context(tc.tile_pool(name="const", bufs=1))
        zpool = ctx.enter_context(tc.tile_pool(name="znat", bufs=2 * TPG))
        ztp = ctx.enter_context(tc.tile_pool(name="zt", bufs=3))
        hpool = ctx.enter_context(tc.tile_pool(name="hs", bufs=2))
        cpool = ctx.enter_context(tc.tile_pool(name="call", bufs=1))
        opool = ctx.enter_context(tc.tile_pool(name="outs", bufs=2))
        sqpool = ctx.enter_context(tc.tile_pool(name="sq", bufs=2))
        lwp = ctx.enter_context(tc.tile_pool(name="lw", bufs=1))
        pt_ps = ctx.enter_context(tc.tile_pool(name="ptp", bufs=2, space="PSUM"))
        h1_ps = ctx.enter_context(tc.tile_pool(name="h1p", bufs=2, space="PSUM"))
        h2_ps = ctx.enter_context(tc.tile_pool(name="h2p", bufs=2, space="PSUM"))
        c_ps = ctx.enter_context(tc.tile_pool(name="cp", bufs=2, space="PSUM"))

        # ---- z loads for group 0 go out first (SWDGE, casting f32->bf16);
        # weight DMAs ride HWDGE queues in parallel. The first two tiles
        # are column-chunked so the PE can start transposing early.
        zn_group = {g: [] for g in range(NG)}

        def load_group(g):
            for q in range(TPG):
                bt = TPG * g + q
                znt = zpool.tile([128, D], BF16, tag="zn", name="zn")
                if bt <= 3:
                    ncks = 3 if bt <= 1 else 2
                    for ck in range(ncks):
                        cs = ck * (D // ncks)
                        nc.gpsimd.dma_start(
                            znt[:, cs:cs + D // ncks],
                            z_d[bt * 128:(bt + 1) * 128, cs:cs + D // ncks],
                        )
                else:
                    nc.gpsimd.dma_start(znt[:], z_d[bt * 128:(bt + 1) * 128, :])
                zn_group[g].append(znt)

        # t-row first: the h1p group openers depend on it
        te = const.tile([1, B], BF16, tag="te")
        nc.gpsimd.dma_start(te[:], tT_d[:])
        ident = const.tile([128, 128], BF16, tag="ident")
        nc.sync.dma_start(ident[:], id_d[:])
        w1s = const.tile([128, NCH, 128], BF16, tag="w1s")
        nc.sync.dma_start(w1s[:], w1m_d[:].rearrange("p (c h) -> p c h", c=NCH))
        w1e = const.tile([1, 128], BF16, tag="w1e")
        nc.sync.dma_start(w1e[:], w1e_d[:])
        b1c = const.tile([H, 1], F32, tag="b1c")
        nc.sync.dma_start(b1c[:], b1c_d[:])

        load_group(0)

        # needed only after mm1 of group 0 -- keep early HBM bandwidth for z
        w2 = const.tile([H, H], F32R, tag="w2")
        nc.sync.dma_start(w2[:], w2_d[:])
        b2c = const.tile([H, 1], F32, tag="b2c")
        nc.sync.dma_start(b2c[:], b2c_d[:])
        w3s = const.tile([H, CD], F32R, tag="w3s")
        nc.sync.dma_start(w3s[:], w3s_d[:])
        b3f = const.tile([128, CD], F32, tag="b3f")
        nc.sync.dma_start(b3f[:], b3f_d[:])

        load_group(1)

        c_all = cpool.tile([128, NBT, CD], F32, tag="c_all")
        s_parts = lwp.tile([128, NBT, 2], F32, tag="s_parts")

        def lambert_and_store(st, cnt):
            """Solve W for tiles [st, st+cnt) via asymptotic series + one
            log-Newton polish step, scale c by -k, DMA out."""
            def lt(nm):
                return lwp.tile([128, cnt], F32, tag=f"{nm}{st}", name=f"{nm}{st}")

            sv = lt("lw_sv")
            nc.vector.tensor_add(
                sv[:], s_parts[:, st:st + cnt, 0], s_parts[:, st:st + cnt, 1]
            )
            sv = sv[:]
            x = lt("lw_x")
            nc.vector.tensor_scalar(x[:], sv, 256.0, 8.0, ALU.mult, ALU.max)
            L1 = lt("lw_L1")
            nc.scalar.activation(L1[:], x[:], AF.Ln)
            L2 = lt("lw_L2")
            nc.scalar.activation(L2[:], L1[:], AF.Ln)
            # w = L1 - L2 + L2/L1 + L2*(L2-2)/(2*L1^2)
            r1 = lt("lw_r1")
            nc.vector.reciprocal(r1[:], L1[:])
            a = lt("lw_a")
            nc.vector.tensor_mul(a[:], L2[:], r1[:])
            w = lt("lw_w")
            nc.vector.tensor_sub(w[:], L1[:], L2[:])
            nc.vector.tensor_add(w[:], w[:], a[:])
            t = lt("lw_t")
            nc.vector.tensor_scalar(t[:], L2[:], -2.0, 0.5, ALU.add, ALU.mult)
            nc.vector.tensor_mul(t[:], t[:], a[:])
            nc.vector.tensor_mul(t[:], t[:], r1[:])
            nc.vector.tensor_add(w[:], w[:], t[:])
            # polish: w -= (ln w + w - ln x) * w / (1 + w)
            g = lt("lw_g")
            wp1 = lt("lw_wp1")
            rcp = lt("lw_rcp")
            nc.scalar.activation(g[:], w[:], AF.Ln)
            nc.vector.tensor_add(g[:], g[:], w[:])
            nc.vector.tensor_sub(g[:], g[:], L1[:])
            nc.vector.tensor_scalar_add(wp1[:], w[:], 1.0)
            nc.vector.reciprocal(rcp[:], wp1[:])
            nc.vector.tensor_mul(g[:], g[:], w[:])
            nc.vector.tensor_mul(g[:], g[:], rcp[:])
            nc.vector.tensor_sub(w[:], w[:], g[:])
            # kneg = -sqrt(256*w/s)  (0 when s == 0: w*rcp(s-guard) ~ 0)
            sg = lt("lw_sg")
            nc.vector.tensor_scalar_max(sg[:], sv, 1e-30)
            nc.vector.reciprocal(rcp[:], sg[:])
            nc.vector.tensor_mul(sg[:], w[:], rcp[:])
            kneg = lt("lw_kneg")
            nc.scalar.activation(kneg[:], sg[:], AF.Sqrt, scale=256.0)
            nc.vector.tensor_scalar_mul(kneg[:], kneg[:], -1.0)
            for i in range(cnt):
                bt = st + i
                ot = opool.tile([128, CD], F32, tag="ot", name="ot")
                nc.vector.tensor_scalar_mul(
                    ot[:], c_all[:, bt, :], kneg[:, i:i + 1]
                )
                eng = nc.sync if bt % 2 == 0 else nc.scalar
                eng.dma_start(out_d[bt * 128:(bt + 1) * 128, :], ot[:])

        def emit_square(bt, nb):
            sq = sqpool.tile([128, 512], F32, tag="sq", name="sq")
            nc.scalar.activation(
                sq[:], c_all[:, bt, nb * 512:(nb + 1) * 512],
                AF.Square, accum_out=s_parts[:, bt, nb:nb + 1],
            )

        def tail_half(g, hf, h1p, defer_squares=False):
            # process quarters [2*hf, 2*hf+1] of group g (h1p is [128, 256])
            h1s = hpool.tile([H, 256], F32R, tag="h1s", name="h1s")
            nc.scalar.activation(h1s[:], h1p[0:H, :], AF.Tanh, bias=b1c[:])
            h2p = h2_ps.tile([H, 256], F32, tag="h2p", name="h2p")
            nc.tensor.matmul(h2p[:], w2[:], h1s[:], start=True, stop=True)
            h2s = hpool.tile([H, 256], F32R, tag="h2s", name="h2s")
            nc.scalar.activation(h2s[:], h2p[:], AF.Tanh, bias=b2c[:])
            for qq in range(2):
                bt = TPG * g + 2 * hf + qq
                for nb in range(2):
                    cp = c_ps.tile([128, 512], F32, tag="cp", name="cp")
                    nc.tensor.matmul(
                        cp[:], h2s[:, qq * 128:(qq + 1) * 128],
                        w3s[:, nb * 512:(nb + 1) * 512],
                        start=True, stop=True,
                    )
                    # c = cp + b3S  (DVE, PSUM -> SBUF), then the half's
                    # squared row-sum immediately (keeps squares off the tail)
                    nc.vector.tensor_add(
                        c_all[:, bt, nb * 512:(nb + 1) * 512],
                        cp[:], b3f[:, nb * 512:(nb + 1) * 512],
                    )
                    if not defer_squares:
                        emit_square(bt, nb)

        # ---- main loop: batch-tile-major z pipeline, half-group tails ----
        for g in range(NG):
            zn = zn_group[g]
            work = [(q, jg) for q in range(TPG) for jg in range(NJG)]
            h1ps = {}
            for hf in range(2):
                h1ps[hf] = h1_ps.tile([128, 256], F32, tag="h1p", name="h1p")
                # t column opens the half's accumulation group
                cst = (g * TPG + 2 * hf) * 128
                nc.tensor.matmul(
                    h1ps[hf][:], w1e[:], te[:, cst:cst + 256],
                    start=True, stop=False,
                )
            pts = {}

            def emit_transpose(idx):
                q, jg = work[idx]
                pt = pt_ps.tile([128, 1024], BF16, tag="pt", name="pt")
                for u in range(8):
                    j = jg * 8 + u
                    nc.tensor.matmul(
                        pt[:, u * 128:(u + 1) * 128],
                        zn[q][:, j * 128:(j + 1) * 128],
                        ident[:],
                        start=(u == 0), stop=(u == 7),
                        is_transpose=True,
                    )
                pts[idx] = pt

            emit_transpose(0)
            for idx, (q, jg) in enumerate(work):
                if idx + 1 < len(work):
                    emit_transpose(idx + 1)  # keep PE one panel ahead
                zt = ztp.tile([128, 1024], BF16, tag="zt", name="zt")
                nc.vector.tensor_copy(zt[:], pts.pop(idx)[:])
                hf, qq = q // 2, q % 2
                lastq = (qq == 1 and jg == NJG - 1)
                for u in range(8):
                    j = jg * 8 + u
                    nc.tensor.matmul(
                        h1ps[hf][:, qq * 128:(qq + 1) * 128],
                        w1s[:, j, :], zt[:, u * 128:(u + 1) * 128],
                        start=False, stop=(lastq and u == 7),
                    )
                if lastq:
                    tail_half(g, hf, h1ps.pop(hf), defer_squares=(g == 1))
                    if g == 0:
                        if hf == 1:
                            lambert_and_store(0, TPG)
                    else:
                        # squares deferred so the critical-path tanh of the
                        # NEXT half is already queued ahead of them on ACT
                        if hf == 1:
                            for bt2 in (TPG, TPG + 1):
                                emit_square(bt2, 0)
                                emit_square(bt2, 1)
                            lambert_and_store(TPG, 2)
                            for bt2 in (TPG + 2, TPG + 3):
                                emit_square(bt2, 0)
                                emit_square(bt2, 1)
                            lambert_and_store(TPG + 2, 2)

    nc.compile()
    return nc


def host_prep(z, t, W1, b1, W2, b2, W3, b3):
    """Host-side weight re-layout + per-core shard maps."""
    f = np.float32
    bf = ml_dtypes.bfloat16
    z = np.asarray(z, f)
    t = np.asarray(t, f)
    W1 = np.asarray(W1, f)
    b1 = np.asarray(b1, f)
    W2 = np.asarray(W2, f)
    b2 = np.asarray(b2, f)
    W3 = np.asarray(W3, f)
    b3 = np.asarray(b3, f)

    # mm1 stationary chunks (bf16, padded to 128 cols for FWL):
    # w1m[p, j*128 + h] = W1[1 + j*128 + p, h]
    w1m = np.zeros((128, NCH, 128), bf)
    w1m[:, :, :H] = W1[1:, :].reshape(NCH, 128, H).transpose(1, 0, 2).astype(bf)
    w1m = np.ascontiguousarray(w1m.reshape(128, NCH * 128))
    w1e = np.zeros((1, 128), bf)
    w1e[0, :H] = W1[0, :].astype(bf)
    b1c = np.ascontiguousarray(b1.reshape(H, 1))
    b2c = np.ascontiguousarray(b2.reshape(H, 1))

    # fold the p -> c map into W3 (and b3)
    W3r = W3.reshape(H, CD // 4, 12)
    W3S = np.empty((H, CD // 4, 4), f)
    W3S[..., 0] = (W3r[..., 6] + W3r[..., 7] + W3r[..., 8]) / MASS
    W3S[..., 1] = W3r[..., 9]
    W3S[..., 2] = W3r[..., 10]
    W3S[..., 3] = W3r[..., 11]
    b3r = b3.reshape(CD // 4, 12)
    b3S = np.empty((CD // 4, 4), f)
    b3S[..., 0] = (b3r[..., 6] + b3r[..., 7] + b3r[..., 8]) / MASS
    b3S[..., 1] = b3r[..., 9]
    b3S[..., 2] = b3r[..., 10]
    b3S[..., 3] = b3r[..., 11]
    w3s = np.ascontiguousarray(W3S.reshape(H, CD))
    b3f = np.ascontiguousarray(np.broadcast_to(b3S.reshape(1, CD), (128, CD)))

    ident = np.eye(128, dtype=bf)

    in_maps = []
    for c in range(N_CORES):
        sl = slice(c * B, (c + 1) * B)
        in_maps.append({
            "z": np.ascontiguousarray(z[sl]),
            "tT": np.ascontiguousarray(t[sl].reshape(1, B)),
            "w1m": w1m,
            "w1e": w1e,
            "b1c": b1c,
            "w2": W2,
            "b2c": b2c,
            "w3s": w3s,
            "b3f": b3f,
            "ident": ident,
        })
    return in_maps


_NC_CACHE = None


def _get_nc():
    global _NC_CACHE
    if _NC_CACHE is None:
        _NC_CACHE = build_kernel()
    return _NC_CACHE


def run(inputs, trace=False):
    """Returns (full_output, BassKernelResults)."""
    nc = _get_nc()
    in_maps = host_prep(**inputs)
    res = run_bass_kernel_spmd(
        nc, in_maps, list(range(N_CORES)), trace=trace,
    )
    out = np.concatenate([r["out"] for r in res.results], axis=0)
    return out.astype(np.float32, copy=False), res


def kernel(**inputs):
    out, _ = run(inputs)
    return out


# revision 21
# speedup vs baseline: 1.3181x; 1.0001x over previous
# Trainium2 Bass kernel for nn_CVXPolicy_MultiQuadcopter.
#
# Math (per sample):
#   x  = concat([t, z])                      (3073,)
#   h1 = tanh(x @ W1 + b1)                   (100,)
#   h2 = tanh(h1 @ W2 + b2)                  (100,)
#   p  = h2 @ W3 + b3                        (3072,)
#   c  = S(p)   (per-agent sparse linear map)   (1024,)
#   s  = ||c||^2 ; w = W(256*s) ; k = sqrt(256*w/s)
#   u* = -k * c
#
# Because c = S(p) is linear in p, S is folded into W3 on the host:
#   c = h2 @ (W3 @ S) + b3 @ S = h2 @ W3S + b3S
# which shrinks the last matmul 3x and removes all on-device shuffles.
#
# Sharding: pure data parallelism. Batch 8192 is split into 8 shards of
# 1024 rows, one per NeuronCore; the tiny MLP weights are replicated.
#
# Device pipeline per core (batch shard B=1024):
#   - z is cast-DMA'd (SWDGE) to bf16 on load; mm1 contracts over the
#     3072 dim, so z tiles are transposed on-chip through the PE
#     (identity matmul, bf16, batched 8 chunks per PSUM bank) and copied
#     to SBUF by the DVE, then consumed as the moving operand of mm1
#     (bf16, FWL weight loads via 128-wide padded W1 chunks). The
#     pipeline is batch-tile-major so compute starts as soon as the
#     first z tile lands.
#   - Layer-1/2 activations are kept transposed ([feature, batch]); those
#     matmuls run in fp32r. b1/b2 are applied as per-partition bias in
#     the tanh activation; tails run per half-group (256 batch) to keep
#     the last tile's critical path short.
#   - mm3 produces c in natural layout [128 b x 1024]; b3S is added as a
#     host-prebroadcast [128, 1024] tile; squared row-sums give s
#     (fused activation accumulate); Lambert-W is solved by an
#     asymptotic series + one log-Newton polish; c is scaled by -k and
#     streamed out. The ACT engine function-table rotation is kept
#     minimal (table loads cost ~1.3us each).

import numpy as np
import ml_dtypes
from contextlib import ExitStack

import concourse.bass as bass
import concourse.tile as tile
from concourse import bacc, mybir
from concourse.bass_utils import run_bass_kernel_spmd

F32 = mybir.dt.float32
F32R = mybir.dt.float32r
BF16 = mybir.dt.bfloat16

N_CORES = 8
BATCH = 8192
B = BATCH // N_CORES      # batch rows per core
D = 3072                  # state dim
H = 100                   # hidden
CD = 1024                 # control dim
NCH = D // 128            # 24 contraction chunks for mm1
NBT = B // 128            # 8 batch tiles per core
GROUP = 512               # batch columns per outer pass
NG = B // GROUP           # 2 groups per core
TPG = GROUP // 128        # 4 batch tiles per group
NJG = NCH // 8            # 3 transpose panels (of 8 chunks) per b-tile
MASS = 0.5

AF = mybir.ActivationFunctionType
ALU = mybir.AluOpType


def build_kernel():
    nc = bacc.Bacc(None, target_bir_lowering=False, enable_partition_id=False)

    z_d = nc.declare_dram_parameter("z", [B, D], F32, isOutput=False)
    tT_d = nc.declare_dram_parameter("tT", [1, B], F32, isOutput=False)
    w1m_d = nc.declare_dram_parameter("w1m", [128, NCH * 128], BF16, isOutput=False)
    w1e_d = nc.declare_dram_parameter("w1e", [1, 128], BF16, isOutput=False)
    b1c_d = nc.declare_dram_parameter("b1c", [H, 1], F32, isOutput=False)
    w2_d = nc.declare_dram_parameter("w2", [H, H], F32R, isOutput=False)
    b2c_d = nc.declare_dram_parameter("b2c", [H, 1], F32, isOutput=False)
    w3s_d = nc.declare_dram_parameter("w3s", [H, CD], F32R, isOutput=False)
    b3f_d = nc.declare_dram_parameter("b3f", [128, CD], F32, isOutput=False)
    id_d = nc.declare_dram_parameter("ident", [128, 128], BF16, isOutput=False)
    out_d = nc.declare_dram_parameter("out", [B, CD], F32, isOutput=True)

    with ExitStack() as ctx:
        tc = ctx.enter_context(tile.TileContext(nc))

        const = ctx.enter_context(tc.tile_pool(name="const", bufs=1))
        zpool = ctx.enter_context(tc.tile_pool(name="znat", bufs=2 * TPG))
        ztp = ctx.enter_context(tc.tile_pool(name="zt", bufs=3))
        hpool = ctx.enter_context(tc.tile_pool(name="hs", bufs=2))
        cpool = ctx.enter_context(tc.tile_pool(name="call", bufs=1))
        opool = ctx.enter_context(tc.tile_pool(name="outs", bufs=2))
        sqpool = ctx.enter_context(tc.tile_pool(name="sq", bufs=2))
        lwp = ctx.enter_context(tc.tile_pool(name="lw", bufs=1))
        pt_ps = ctx.enter_context(tc.tile_pool(name="ptp", bufs=2, space="PSUM"))
        h1_ps = ctx.enter_context(tc.tile_pool(name="h1p", bufs=2, space="PSUM"))
        h2_ps = ctx.enter_context(tc.tile_pool(name="h2p", bufs=2, space="PSUM"))
        c_ps = ctx.enter_context(tc.tile_pool(name="cp", bufs=2, space="PSUM"))

        # ---- z loads for group 0 go out first (SWDGE, casting f32->bf16);
        # weight DMAs ride HWDGE queues in parallel. The first two tiles
        # are column-chunked so the PE can start transposing early.
        zn_group = {g: [] for g in range(NG)}

        def load_group(g):
            for q in range(TPG):
                bt = TPG * g + q
                znt = zpool.tile([128, D], BF16, tag="zn", name="zn")
                if bt <= 3:
                    ncks = 3 if bt <= 1 else 2
                    for ck in range(ncks):
                        cs = ck * (D // ncks)
                        nc.gpsimd.dma_start(
                            znt[:, cs:cs + D // ncks],
                            z_d[bt * 128:(bt + 1) * 128, cs:cs + D // ncks],
                        )
                else:
                    nc.gpsimd.dma_start(znt[:], z_d[bt * 128:(bt + 1) * 128, :])
                zn_group[g].append(znt)

        # t-row first: the h1p group openers depend on it
        te = const.tile([1, B], BF16, tag="te")
        nc.gpsimd.dma_start(te[:], tT_d[:])
        ident = const.tile([128, 128], BF16, tag="ident")
        nc.sync.dma_start(ident[:], id_d[:])
        w1s = const.tile([128, NCH, 128], BF16, tag="w1s")
        nc.sync.dma_start(w1s[:], w1m_d[:].rearrange("p (c h) -> p c h", c=NCH))
        w1e = const.tile([1, 128], BF16, tag="w1e")
        nc.sync.dma_start(w1e[:], w1e_d[:])
        b1c = const.tile([H, 1], F32, tag="b1c")
        nc.sync.dma_start(b1c[:], b1c_d[:])

        load_group(0)

        # needed only after mm1 of group 0 -- keep early HBM bandwidth for z
        w2 = const.tile([H, H], F32R, tag="w2")
        nc.sync.dma_start(w2[:], w2_d[:])
        b2c = const.tile([H, 1], F32, tag="b2c")
        nc.sync.dma_start(b2c[:], b2c_d[:])
        w3s = const.tile([H, CD], F32R, tag="w3s")
        nc.sync.dma_start(w3s[:], w3s_d[:])
        b3f = const.tile([128, CD], F32, tag="b3f")
        nc.sync.dma_start(b3f[:], b3f_d[:])

        load_group(1)

        c_all = cpool.tile([128, NBT, CD], F32, tag="c_all")
        s_parts = lwp.tile([128, NBT, 2], F32, tag="s_parts")

        def lambert_and_store(st, cnt):
            """Solve W for tiles [st, st+cnt) via asymptotic series + one
            log-Newton polish step, scale c by -k, DMA out."""
            def lt(nm):
                return lwp.tile([128, cnt], F32, tag=f"{nm}{st}", name=f"{nm}{st}")

            sv = lt("lw_sv")
            nc.vector.tensor_add(
                sv[:], s_parts[:, st:st + cnt, 0], s_parts[:, st:st + cnt, 1]
            )
            sv = sv[:]
            x = lt("lw_x")
            nc.vector.tensor_scalar(x[:], sv, 256.0, 8.0, ALU.mult, ALU.max)
            L1 = lt("lw_L1")
            nc.scalar.activation(L1[:], x[:], AF.Ln)
            L2 = lt("lw_L2")
            nc.scalar.activation(L2[:], L1[:], AF.Ln)
            # w = L1 - L2 + L2/L1 + L2*(L2-2)/(2*L1^2)
            r1 = lt("lw_r1")
            nc.vector.reciprocal(r1[:], L1[:])
            a = lt("lw_a")
            nc.vector.tensor_mul(a[:], L2[:], r1[:])
            w = lt("lw_w")
            nc.vector.tensor_sub(w[:], L1[:], L2[:])
            nc.vector.tensor_add(w[:], w[:], a[:])
            t = lt("lw_t")
            nc.vector.tensor_scalar(t[:], L2[:], -2.0, 0.5, ALU.add, ALU.mult)
            nc.vector.tensor_mul(t[:], t[:], a[:])
            nc.vector.tensor_mul(t[:], t[:], r1[:])
            nc.vector.tensor_add(w[:], w[:], t[:])
            # polish: w -= (ln w + w - ln x) * w / (1 + w)
            g = lt("lw_g")
            wp1 = lt("lw_wp1")
            rcp = lt("lw_rcp")
            nc.scalar.activation(g[:], w[:], AF.Ln)
            nc.vector.tensor_add(g[:], g[:], w[:])
            nc.vector.tensor_sub(g[:], g[:], L1[:])
            nc.vector.tensor_scalar_add(wp1[:], w[:], 1.0)
            nc.vector.reciprocal(rcp[:], wp1[:])
            nc.vector.tensor_mul(g[:], g[:], w[:])
            nc.vector.tensor_mul(g[:], g[:], rcp[:])
            nc.vector.tensor_sub(w[:], w[:], g[:])
            # kneg = -sqrt(256*w/s)  (0 when s == 0: w*rcp(s-guard) ~ 0)
            sg = lt("lw_sg")
            nc.vector.tensor_scalar_max(sg[:], sv, 1e-30)
            nc.vector.reciprocal(rcp[:], sg[:])
            nc.vector.tensor_mul(sg[:], w[:], rcp[:])
            kneg = lt("lw_kneg")
            nc.scalar.activation(kneg[:], sg[:], AF.Sqrt, scale=256.0)
            nc.vector.tensor_scalar_mul(kneg[:], kneg[:], -1.0)
            for i in range(cnt):
                bt = st + i
                ot = opool.tile([128, CD], F32, tag="ot", name="ot")
                nc.vector.tensor_scalar_mul(
                    ot[:], c_all[:, bt, :], kneg[:, i:i + 1]
                )
                nc.sync.dma_start(out_d[bt * 128:(bt + 1) * 128, :], ot[:])

        def emit_square(bt, nb):
            sq = sqpool.tile([128, 512], F32, tag="sq", name="sq")
            nc.scalar.activation(
                sq[:], c_all[:, bt, nb * 512:(nb + 1) * 512],
                AF.Square, accum_out=s_parts[:, bt, nb:nb + 1],
            )

        def tail_half(g, hf, h1p, defer_squares=False):
            # process quarters [2*hf, 2*hf+1] of group g (h1p is [128, 256])
            h1s = hpool.tile([H, 256], F32R, tag="h1s", name="h1s")
            nc.scalar.activation(h1s[:], h1p[0:H, :], AF.Tanh, bias=b1c[:])
            h2p = h2_ps.tile([H, 256], F32, tag="h2p", name="h2p")
            nc.tensor.matmul(h2p[:], w2[:], h1s[:], start=True, stop=True)
            h2s = hpool.tile([H, 256], F32R, tag="h2s", name="h2s")
            nc.scalar.activation(h2s[:], h2p[:], AF.Tanh, bias=b2c[:])
            for qq in range(2):
                bt = TPG * g + 2 * hf + qq
                for nb in range(2):
                    cp = c_ps.tile([128, 512], F32, tag="cp", name="cp")
                    nc.tensor.matmul(
                        cp[:], h2s[:, qq * 128:(qq + 1) * 128],
                        w3s[:, nb * 512:(nb + 1) * 512],
                        start=True, stop=True,
                    )
                    # c = cp + b3S  (DVE, PSUM -> SBUF), then the half's
                    # squared row-sum immediately (keeps squares off the tail)
                    nc.vector.tensor_add(
                        c_all[:, bt, nb * 512:(nb + 1) * 512],
                        cp[:], b3f[:, nb * 512:(nb + 1) * 512],
                    )
                    if not defer_squares:
                        emit_square(bt, nb)

        # ---- main loop: batch-tile-major z pipeline, half-group tails ----
        for g in range(NG):
            zn = zn_group[g]
            work = [(q, jg) for q in range(TPG) for jg in range(NJG)]
            h1ps = {}
            for hf in range(2):
                h1ps[hf] = h1_ps.tile([128, 256], F32, tag="h1p", name="h1p")
                # t column opens the half's accumulation group
                cst = (g * TPG + 2 * hf) * 128
                nc.tensor.matmul(
                    h1ps[hf][:], w1e[:], te[:, cst:cst + 256],
                    start=True, stop=False,
                )
            pts = {}

            def emit_transpose(idx):
                q, jg = work[idx]
                pt = pt_ps.tile([128, 1024], BF16, tag="pt", name="pt")
                for u in range(8):
                    j = jg * 8 + u
                    nc.tensor.matmul(
                        pt[:, u * 128:(u + 1) * 128],
                        zn[q][:, j * 128:(j + 1) * 128],
                        ident[:],
                        start=(u == 0), stop=(u == 7),
                        is_transpose=True,
                    )
                pts[idx] = pt

            emit_transpose(0)
            for idx, (q, jg) in enumerate(work):
                if idx + 1 < len(work):
                    emit_transpose(idx + 1)  # keep PE one panel ahead
                zt = ztp.tile([128, 1024], BF16, tag="zt", name="zt")
                nc.vector.tensor_copy(zt[:], pts.pop(idx)[:])
                hf, qq = q // 2, q % 2
                lastq = (qq == 1 and jg == NJG - 1)
                for u in range(8):
                    j = jg * 8 + u
                    nc.tensor.matmul(
                        h1ps[hf][:, qq * 128:(qq + 1) * 128],
                        w1s[:, j, :], zt[:, u * 128:(u + 1) * 128],
                        start=False, stop=(lastq and u == 7),
                    )
                if lastq:
                    tail_half(g, hf, h1ps.pop(hf), defer_squares=(g == 1))
                    if g == 0:
                        if hf == 1:
                            lambert_and_store(0, TPG)
                    else:
                        # squares deferred so the critical-path tanh of the
                        # NEXT half is already queued ahead of them on ACT
                        if hf == 1:
                            for bt2 in (TPG, TPG + 1):
                                emit_square(bt2, 0)
                                emit_square(bt2, 1)
                            lambert_and_store(TPG, 2)
                            for bt2 in (TPG + 2, TPG + 3):
                                emit_square(bt2, 0)
                                emit_square(bt2, 1)
                            lambert_and_store(TPG + 2, 2)

    nc.compile()
    return nc


def host_prep(z, t, W1, b1, W2, b2, W3, b3):
    """Host-side weight re-layout + per-core shard maps."""
    f = np.float32
    bf = ml_dtypes.bfloat16
    z = np.asarray(z, f)
    t = np.asarray(t, f)
    W1 = np.asarray(W1, f)
    b1 = np.asarray(b1, f)
    W2 = np.asarray(W2, f)
    b2 = np.asarray(b2, f)
    W3 = np.asarray(W3, f)
    b3 = np.asarray(b3, f)

    # mm1 stationary chunks (bf16, padded to 128 cols for FWL):
    # w1m[p, j*128 + h] = W1[1 + j*128 + p, h]
    w1m = np.zeros((128, NCH, 128), bf)
    w1m[:, :, :H] = W1[1:, :].reshape(NCH, 128, H).transpose(1, 0, 2).astype(bf)
    w1m = np.ascontiguousarray(w1m.reshape(128, NCH * 128))
    w1e = np.zeros((1, 128), bf)
    w1e[0, :H] = W1[0, :].astype(bf)
    b1c = np.ascontiguousarray(b1.reshape(H, 1))
    b2c = np.ascontiguousarray(b2.reshape(H, 1))

    # fold the p -> c map into W3 (and b3)
    W3r = W3.reshape(H, CD // 4, 12)
    W3S = np.empty((H, CD // 4, 4), f)
    W3S[..., 0] = (W3r[..., 6] + W3r[..., 7] + W3r[..., 8]) / MASS
    W3S[..., 1] = W3r[..., 9]
    W3S[..., 2] = W3r[..., 10]
    W3S[..., 3] = W3r[..., 11]
    b3r = b3.reshape(CD // 4, 12)
    b3S = np.empty((CD // 4, 4), f)
    b3S[..., 0] = (b3r[..., 6] + b3r[..., 7] + b3r[..., 8]) / MASS
    b3S[..., 1] = b3r[..., 9]
    b3S[..., 2] = b3r[..., 10]
    b3S[..., 3] = b3r[..., 11]
    w3s = np.ascontiguousarray(W3S.reshape(H, CD))
    b3f = np.ascontiguousarray(np.broadcast_to(b3S.reshape(1, CD), (128, CD)))

    ident = np.eye(128, dtype=bf)

    in_maps = []
    for c in range(N_CORES):
        sl = slice(c * B, (c + 1) * B)
        in_maps.append({
            "z": np.ascontiguousarray(z[sl]),
            "tT": np.ascontiguousarray(t[sl].reshape(1, B)),
            "w1m": w1m,
            "w1e": w1e,
            "b1c": b1c,
            "w2": W2,
            "b2c": b2c,
            "w3s": w3s,
            "b3f": b3f,
            "ident": ident,
        })
    return in_maps


_NC_CACHE = None


def _get_nc():
    global _NC_CACHE
    if _NC_CACHE is None:
        _NC_CACHE = build_kernel()
    return _NC_CACHE


def run(inputs, trace=False):
    """Returns (full_output, BassKernelResults)."""
    nc = _get_nc()
    in_maps = host_prep(**inputs)
    res = run_bass_kernel_spmd(
        nc, in_maps, list(range(N_CORES)), trace=trace,
    )
    out = np.concatenate([r["out"] for r in res.results], axis=0)
    return out.astype(np.float32, copy=False), res


def kernel(**inputs):
    out, _ = run(inputs)
    return out


# revision 22
# speedup vs baseline: 1.3505x; 1.0246x over previous
# Trainium2 Bass kernel for nn_CVXPolicy_MultiQuadcopter.
#
# Math (per sample):
#   x  = concat([t, z])                      (3073,)
#   h1 = tanh(x @ W1 + b1)                   (100,)
#   h2 = tanh(h1 @ W2 + b2)                  (100,)
#   p  = h2 @ W3 + b3                        (3072,)
#   c  = S(p)   (per-agent sparse linear map)   (1024,)
#   s  = ||c||^2 ; w = W(256*s) ; k = sqrt(256*w/s)
#   u* = -k * c
#
# Because c = S(p) is linear in p, S is folded into W3 on the host:
#   c = h2 @ (W3 @ S) + b3 @ S = h2 @ W3S + b3S
# which shrinks the last matmul 3x and removes all on-device shuffles.
#
# Sharding: pure data parallelism. Batch 8192 is split into 8 shards of
# 1024 rows, one per NeuronCore; the tiny MLP weights are replicated.
#
# Device pipeline per core (batch shard B=1024):
#   - z is cast-DMA'd (SWDGE) to bf16 on load; mm1 contracts over the
#     3072 dim, so z tiles are transposed on-chip through the PE
#     (identity matmul, bf16, batched 8 chunks per PSUM bank) and copied
#     to SBUF by the DVE, then consumed as the moving operand of mm1
#     (bf16, FWL weight loads via 128-wide padded W1 chunks). The
#     pipeline is batch-tile-major so compute starts as soon as the
#     first z tile lands.
#   - Layer-1/2 activations are kept transposed ([feature, batch]); those
#     matmuls run in fp32r. b1/b2 are applied as per-partition bias in
#     the tanh activation; tails run per half-group (256 batch) to keep
#     the last tile's critical path short.
#   - mm3 produces c in natural layout [128 b x 1024]; b3S is added as a
#     host-prebroadcast [128, 1024] tile; squared row-sums give s
#     (fused activation accumulate); Lambert-W is solved by an
#     asymptotic series + one log-Newton polish; c is scaled by -k and
#     streamed out. The ACT engine function-table rotation is kept
#     minimal (table loads cost ~1.3us each).

import numpy as np
import ml_dtypes
from contextlib import ExitStack

import concourse.bass as bass
import concourse.tile as tile
from concourse import bacc, mybir
from concourse.bass_utils import run_bass_kernel_spmd

F32 = mybir.dt.float32
F32R = mybir.dt.float32r
BF16 = mybir.dt.bfloat16

N_CORES = 8
BATCH = 8192
B = BATCH // N_CORES      # batch rows per core
D = 3072                  # state dim
H = 100                   # hidden
CD = 1024                 # control dim
NCH = D // 128            # 24 contraction chunks for mm1
NBT = B // 128            # 8 batch tiles per core
GROUP = 512               # batch columns per outer pass
NG = B // GROUP           # 2 groups per core
TPG = GROUP // 128        # 4 batch tiles per group
NJG = NCH // 8            # 3 transpose panels (of 8 chunks) per b-tile
MASS = 0.5

AF = mybir.ActivationFunctionType
ALU = mybir.AluOpType


def build_kernel():
    nc = bacc.Bacc(None, target_bir_lowering=False, enable_partition_id=False)

    z_d = nc.declare_dram_parameter("z", [B, D], F32, isOutput=False)
    tT_d = nc.declare_dram_parameter("tT", [1, B], F32, isOutput=False)
    w1m_d = nc.declare_dram_parameter("w1m", [128, NCH * 128], BF16, isOutput=False)
    w1e_d = nc.declare_dram_parameter("w1e", [1, 128], BF16, isOutput=False)
    b1c_d = nc.declare_dram_parameter("b1c", [H, 1], F32, isOutput=False)
    w2_d = nc.declare_dram_parameter("w2", [H, H], F32R, isOutput=False)
    b2c_d = nc.declare_dram_parameter("b2c", [H, 1], F32, isOutput=False)
    w3s_d = nc.declare_dram_parameter("w3s", [H, CD], F32R, isOutput=False)
    b3f_d = nc.declare_dram_parameter("b3f", [128, CD], F32, isOutput=False)
    id_d = nc.declare_dram_parameter("ident", [128, 128], BF16, isOutput=False)
    out_d = nc.declare_dram_parameter("out", [B, CD], F32, isOutput=True)

    with ExitStack() as ctx:
        tc = ctx.enter_context(tile.TileContext(nc))

        const = ctx.enter_context(tc.tile_pool(name="const", bufs=1))
        zpool = ctx.enter_context(tc.tile_pool(name="znat", bufs=2 * TPG))
        ztp = ctx.enter_context(tc.tile_pool(name="zt", bufs=3))
        hpool = ctx.enter_context(tc.tile_pool(name="hs", bufs=2))
        cpool = ctx.enter_context(tc.tile_pool(name="call", bufs=1))
        opool = ctx.enter_context(tc.tile_pool(name="outs", bufs=2))
        sqpool = ctx.enter_context(tc.tile_pool(name="sq", bufs=2))
        lwp = ctx.enter_context(tc.tile_pool(name="lw", bufs=1))
        pt_ps = ctx.enter_context(tc.tile_pool(name="ptp", bufs=2, space="PSUM"))
        h1_ps = ctx.enter_context(tc.tile_pool(name="h1p", bufs=2, space="PSUM"))
        h2_ps = ctx.enter_context(tc.tile_pool(name="h2p", bufs=2, space="PSUM"))
        c_ps = ctx.enter_context(tc.tile_pool(name="cp", bufs=2, space="PSUM"))

        # ---- z loads for group 0 go out first (SWDGE, casting f32->bf16);
        # weight DMAs ride HWDGE queues in parallel. The first two tiles
        # are column-chunked so the PE can start transposing early.
        zn_group = {g: [] for g in range(NG)}

        def load_group(g):
            for q in range(TPG):
                bt = TPG * g + q
                znt = zpool.tile([128, D], BF16, tag="zn", name="zn")
                ncks = 3 if bt <= 1 else 2
                for ck in range(ncks):
                    cs = ck * (D // ncks)
                    nc.gpsimd.dma_start(
                        znt[:, cs:cs + D // ncks],
                        z_d[bt * 128:(bt + 1) * 128, cs:cs + D // ncks],
                    )
                zn_group[g].append(znt)

        # t-row first: the h1p group openers depend on it
        te = const.tile([1, B], BF16, tag="te")
        nc.gpsimd.dma_start(te[:], tT_d[:])
        ident = const.tile([128, 128], BF16, tag="ident")
        nc.sync.dma_start(ident[:], id_d[:])
        w1s = const.tile([128, NCH, 128], BF16, tag="w1s")
        nc.sync.dma_start(w1s[:], w1m_d[:].rearrange("p (c h) -> p c h", c=NCH))
        w1e = const.tile([1, 128], BF16, tag="w1e")
        nc.sync.dma_start(w1e[:], w1e_d[:])
        b1c = const.tile([H, 1], F32, tag="b1c")
        nc.sync.dma_start(b1c[:], b1c_d[:])

        load_group(0)

        # needed only after mm1 of group 0 -- keep early HBM bandwidth for z
        w2 = const.tile([H, H], F32R, tag="w2")
        nc.sync.dma_start(w2[:], w2_d[:])
        b2c = const.tile([H, 1], F32, tag="b2c")
        nc.sync.dma_start(b2c[:], b2c_d[:])
        w3s = const.tile([H, CD], F32R, tag="w3s")
        nc.sync.dma_start(w3s[:], w3s_d[:])
        b3f = const.tile([128, CD], F32, tag="b3f")
        nc.sync.dma_start(b3f[:], b3f_d[:])

        load_group(1)

        c_all = cpool.tile([128, NBT, CD], F32, tag="c_all")
        s_parts = lwp.tile([128, NBT, 2], F32, tag="s_parts")

        def lambert_and_store(st, cnt):
            """Solve W for tiles [st, st+cnt) via asymptotic series + one
            log-Newton polish step, scale c by -k, DMA out."""
            def lt(nm):
                return lwp.tile([128, cnt], F32, tag=f"{nm}{st}", name=f"{nm}{st}")

            sv = lt("lw_sv")
            nc.vector.tensor_add(
                sv[:], s_parts[:, st:st + cnt, 0], s_parts[:, st:st + cnt, 1]
            )
            sv = sv[:]
            x = lt("lw_x")
            nc.vector.tensor_scalar(x[:], sv, 256.0, 8.0, ALU.mult, ALU.max)
            L1 = lt("lw_L1")
            nc.scalar.activation(L1[:], x[:], AF.Ln)
            L2 = lt("lw_L2")
            nc.scalar.activation(L2[:], L1[:], AF.Ln)
            # w = L1 - L2 + L2/L1 + L2*(L2-2)/(2*L1^2)
            r1 = lt("lw_r1")
            nc.vector.reciprocal(r1[:], L1[:])
            a = lt("lw_a")
            nc.vector.tensor_mul(a[:], L2[:], r1[:])
            w = lt("lw_w")
            nc.vector.tensor_sub(w[:], L1[:], L2[:])
            nc.vector.tensor_add(w[:], w[:], a[:])
            t = lt("lw_t")
            nc.vector.tensor_scalar(t[:], L2[:], -2.0, 0.5, ALU.add, ALU.mult)
            nc.vector.tensor_mul(t[:], t[:], a[:])
            nc.vector.tensor_mul(t[:], t[:], r1[:])
            nc.vector.tensor_add(w[:], w[:], t[:])
            # k = sqrt(256*w/s)  (0 when s == 0: w*rcp(s-guard) ~ 0);
            # the series alone is ~1e-4 accurate -- far below the bf16 floor
            sg = lt("lw_sg")
            rcp = lt("lw_rcp")
            nc.vector.tensor_scalar_max(sg[:], sv, 1e-30)
            nc.vector.reciprocal(rcp[:], sg[:])
            nc.vector.tensor_mul(sg[:], w[:], rcp[:])
            kpos = lt("lw_kpos")
            nc.scalar.activation(kpos[:], sg[:], AF.Sqrt, scale=256.0)
            for i in range(cnt):
                bt = st + i
                ot = opool.tile([128, CD], F32, tag="ot", name="ot")
                nc.vector.tensor_scalar(
                    ot[:], c_all[:, bt, :], kpos[:, i:i + 1], -1.0,
                    ALU.mult, ALU.mult,
                )
                nc.sync.dma_start(out_d[bt * 128:(bt + 1) * 128, :], ot[:])

        def emit_square(bt, nb):
            sq = sqpool.tile([128, 512], F32, tag="sq", name="sq")
            nc.scalar.activation(
                sq[:], c_all[:, bt, nb * 512:(nb + 1) * 512],
                AF.Square, accum_out=s_parts[:, bt, nb:nb + 1],
            )

        def tail_half(g, hf, h1p, defer_squares=False):
            # process quarters [2*hf, 2*hf+1] of group g (h1p is [128, 256])
            h1s = hpool.tile([H, 256], F32R, tag="h1s", name="h1s")
            nc.scalar.activation(h1s[:], h1p[0:H, :], AF.Tanh, bias=b1c[:])
            h2p = h2_ps.tile([H, 256], F32, tag="h2p", name="h2p")
            nc.tensor.matmul(h2p[:], w2[:], h1s[:], start=True, stop=True)
            h2s = hpool.tile([H, 256], F32R, tag="h2s", name="h2s")
            nc.scalar.activation(h2s[:], h2p[:], AF.Tanh, bias=b2c[:])
            for qq in range(2):
                bt = TPG * g + 2 * hf + qq
                for nb in range(2):
                    cp = c_ps.tile([128, 512], F32, tag="cp", name="cp")
                    nc.tensor.matmul(
                        cp[:], h2s[:, qq * 128:(qq + 1) * 128],
                        w3s[:, nb * 512:(nb + 1) * 512],
                        start=True, stop=True,
                    )
                    # c = cp + b3S  (DVE, PSUM -> SBUF), then the half's
                    # squared row-sum immediately (keeps squares off the tail)
                    nc.vector.tensor_add(
                        c_all[:, bt, nb * 512:(nb + 1) * 512],
                        cp[:], b3f[:, nb * 512:(nb + 1) * 512],
                    )
                    if not defer_squares:
                        emit_square(bt, nb)

        # ---- main loop: batch-tile-major z pipeline, half-group tails ----
        for g in range(NG):
            zn = zn_group[g]
            work = [(q, jg) for q in range(TPG) for jg in range(NJG)]
            h1ps = {}
            for hf in range(2):
                h1ps[hf] = h1_ps.tile([128, 256], F32, tag="h1p", name="h1p")
                # t column opens the half's accumulation group
                cst = (g * TPG + 2 * hf) * 128
                nc.tensor.matmul(
                    h1ps[hf][:], w1e[:], te[:, cst:cst + 256],
                    start=True, stop=False,
                )
            pts = {}

            def emit_transpose(idx):
                q, jg = work[idx]
                pt = pt_ps.tile([128, 1024], BF16, tag="pt", name="pt")
                for u in range(8):
                    j = jg * 8 + u
                    nc.tensor.matmul(
                        pt[:, u * 128:(u + 1) * 128],
                        zn[q][:, j * 128:(j + 1) * 128],
                        ident[:],
                        start=(u == 0), stop=(u == 7),
                        is_transpose=True,
                    )
                pts[idx] = pt

            emit_transpose(0)
            for idx, (q, jg) in enumerate(work):
                if idx + 1 < len(work):
                    emit_transpose(idx + 1)  # keep PE one panel ahead
                zt = ztp.tile([128, 1024], BF16, tag="zt", name="zt")
                nc.vector.tensor_copy(zt[:], pts.pop(idx)[:])
                hf, qq = q // 2, q % 2
                lastq = (qq == 1 and jg == NJG - 1)
                for u in range(8):
                    j = jg * 8 + u
                    nc.tensor.matmul(
                        h1ps[hf][:, qq * 128:(qq + 1) * 128],
                        w1s[:, j, :], zt[:, u * 128:(u + 1) * 128],
                        start=False, stop=(lastq and u == 7),
                    )
                if lastq:
                    tail_half(g, hf, h1ps.pop(hf), defer_squares=(g == 1))
                    if g == 0:
                        if hf == 1:
                            lambert_and_store(0, TPG)
                    else:
                        # squares deferred so the critical-path tanh of the
                        # NEXT half is already queued ahead of them on ACT
                        if hf == 1:
                            for bt2 in (TPG, TPG + 1):
                                emit_square(bt2, 0)
                                emit_square(bt2, 1)
                            lambert_and_store(TPG, 2)
                            for bt2 in (TPG + 2, TPG + 3):
                                emit_square(bt2, 0)
                                emit_square(bt2, 1)
                            lambert_and_store(TPG + 2, 2)

    nc.compile()
    return nc


def host_prep(z, t, W1, b1, W2, b2, W3, b3):
    """Host-side weight re-layout + per-core shard maps."""
    f = np.float32
    bf = ml_dtypes.bfloat16
    z = np.asarray(z, f)
    t = np.asarray(t, f)
    W1 = np.asarray(W1, f)
    b1 = np.asarray(b1, f)
    W2 = np.asarray(W2, f)
    b2 = np.asarray(b2, f)
    W3 = np.asarray(W3, f)
    b3 = np.asarray(b3, f)

    # mm1 stationary chunks (bf16, padded to 128 cols for FWL):
    # w1m[p, j*128 + h] = W1[1 + j*128 + p, h]
    w1m = np.zeros((128, NCH, 128), bf)
    w1m[:, :, :H] = W1[1:, :].reshape(NCH, 128, H).transpose(1, 0, 2).astype(bf)
    w1m = np.ascontiguousarray(w1m.reshape(128, NCH * 128))
    w1e = np.zeros((1, 128), bf)
    w1e[0, :H] = W1[0, :].astype(bf)
    b1c = np.ascontiguousarray(b1.reshape(H, 1))
    b2c = np.ascontiguousarray(b2.reshape(H, 1))

    # fold the p -> c map into W3 (and b3)
    W3r = W3.reshape(H, CD // 4, 12)
    W3S = np.empty((H, CD // 4, 4), f)
    W3S[..., 0] = (W3r[..., 6] + W3r[..., 7] + W3r[..., 8]) / MASS
    W3S[..., 1] = W3r[..., 9]
    W3S[..., 2] = W3r[..., 10]
    W3S[..., 3] = W3r[..., 11]
    b3r = b3.reshape(CD // 4, 12)
    b3S = np.empty((CD // 4, 4), f)
    b3S[..., 0] = (b3r[..., 6] + b3r[..., 7] + b3r[..., 8]) / MASS
    b3S[..., 1] = b3r[..., 9]
    b3S[..., 2] = b3r[..., 10]
    b3S[..., 3] = b3r[..., 11]
    w3s = np.ascontiguousarray(W3S.reshape(H, CD))
    b3f = np.ascontiguousarray(np.broadcast_to(b3S.reshape(1, CD), (128, CD)))

    ident = np.eye(128, dtype=bf)

    in_maps = []
    for c in range(N_CORES):
        sl = slice(c * B, (c + 1) * B)
        in_maps.append({
            "z": np.ascontiguousarray(z[sl]),
            "tT": np.ascontiguousarray(t[sl].reshape(1, B)),
            "w1m": w1m,
            "w1e": w1e,
            "b1c": b1c,
            "w2": W2,
            "b2c": b2c,
            "w3s": w3s,
            "b3f": b3f,
            "ident": ident,
        })
    return in_maps


_NC_CACHE = None


def _get_nc():
    global _NC_CACHE
    if _NC_CACHE is None:
        _NC_CACHE = build_kernel()
    return _NC_CACHE


def run(inputs, trace=False):
    """Returns (full_output, BassKernelResults)."""
    nc = _get_nc()
    in_maps = host_prep(**inputs)
    res = run_bass_kernel_spmd(
        nc, in_maps, list(range(N_CORES)), trace=trace,
    )
    out = np.concatenate([r["out"] for r in res.results], axis=0)
    return out.astype(np.float32, copy=False), res


def kernel(**inputs):
    out, _ = run(inputs)
    return out


# revision 23
# speedup vs baseline: 1.4020x; 1.0381x over previous
# Trainium2 Bass kernel for nn_CVXPolicy_MultiQuadcopter.
#
# Math (per sample):
#   x  = concat([t, z])                      (3073,)
#   h1 = tanh(x @ W1 + b1)                   (100,)
#   h2 = tanh(h1 @ W2 + b2)                  (100,)
#   p  = h2 @ W3 + b3                        (3072,)
#   c  = S(p)   (per-agent sparse linear map)   (1024,)
#   s  = ||c||^2 ; w = W(256*s) ; k = sqrt(256*w/s)
#   u* = -k * c
#
# Because c = S(p) is linear in p, S is folded into W3 on the host:
#   c = h2 @ (W3 @ S) + b3 @ S = h2 @ W3S + b3S
# which shrinks the last matmul 3x and removes all on-device shuffles.
#
# Sharding: pure data parallelism. Batch 8192 is split into 8 shards of
# 1024 rows, one per NeuronCore; the tiny MLP weights are replicated.
#
# Device pipeline per core (batch shard B=1024):
#   - z is cast-DMA'd (SWDGE) to bf16 on load; mm1 contracts over the
#     3072 dim, so z tiles are transposed on-chip through the PE
#     (identity matmul, bf16, batched 8 chunks per PSUM bank) and copied
#     to SBUF by the DVE, then consumed as the moving operand of mm1
#     (bf16, FWL weight loads via 128-wide padded W1 chunks). The
#     pipeline is batch-tile-major so compute starts as soon as the
#     first z tile lands.
#   - Layer-1/2 activations are kept transposed ([feature, batch]); those
#     matmuls run in fp32r. b1/b2 are applied as per-partition bias in
#     the tanh activation; tails run per half-group (256 batch) to keep
#     the last tile's critical path short.
#   - mm3 produces c in natural layout [128 b x 1024]; b3S is added as a
#     host-prebroadcast [128, 1024] tile; squared row-sums give s
#     (fused activation accumulate); Lambert-W is solved by an
#     asymptotic series + one log-Newton polish; c is scaled by -k and
#     streamed out. The ACT engine function-table rotation is kept
#     minimal (table loads cost ~1.3us each).

import numpy as np
import ml_dtypes
from contextlib import ExitStack

import concourse.bass as bass
import concourse.tile as tile
from concourse import bacc, mybir
from concourse.bass_utils import run_bass_kernel_spmd

F32 = mybir.dt.float32
F32R = mybir.dt.float32r
BF16 = mybir.dt.bfloat16

N_CORES = 8
BATCH = 8192
B = BATCH // N_CORES      # batch rows per core
D = 3072                  # state dim
H = 100                   # hidden
CD = 1024                 # control dim
NCH = D // 128            # 24 contraction chunks for mm1
NBT = B // 128            # 8 batch tiles per core
GROUP = 512               # batch columns per outer pass
NG = B // GROUP           # 2 groups per core
TPG = GROUP // 128        # 4 batch tiles per group
NJG = NCH // 8            # 3 transpose panels (of 8 chunks) per b-tile
MASS = 0.5

AF = mybir.ActivationFunctionType
ALU = mybir.AluOpType


def build_kernel():
    nc = bacc.Bacc(None, target_bir_lowering=False, enable_partition_id=False)

    z_d = nc.declare_dram_parameter("z", [B, D], F32, isOutput=False)
    tT_d = nc.declare_dram_parameter("tT", [1, B], F32, isOutput=False)
    w1m_d = nc.declare_dram_parameter("w1m", [128, NCH * 128], BF16, isOutput=False)
    w1e_d = nc.declare_dram_parameter("w1e", [1, 128], BF16, isOutput=False)
    b1c_d = nc.declare_dram_parameter("b1c", [H, 1], F32, isOutput=False)
    w2_d = nc.declare_dram_parameter("w2", [H, H], F32R, isOutput=False)
    b2c_d = nc.declare_dram_parameter("b2c", [H, 1], F32, isOutput=False)
    w3s_d = nc.declare_dram_parameter("w3s", [H, CD], F32R, isOutput=False)
    b3f_d = nc.declare_dram_parameter("b3f", [128, CD], F32, isOutput=False)
    id_d = nc.declare_dram_parameter("ident", [128, 128], BF16, isOutput=False)
    out_d = nc.declare_dram_parameter("out", [B, CD], F32, isOutput=True)

    with ExitStack() as ctx:
        tc = ctx.enter_context(tile.TileContext(nc))

        const = ctx.enter_context(tc.tile_pool(name="const", bufs=1))
        zpool = ctx.enter_context(tc.tile_pool(name="znat", bufs=2 * TPG))
        ztp = ctx.enter_context(tc.tile_pool(name="zt", bufs=3))
        hpool = ctx.enter_context(tc.tile_pool(name="hs", bufs=2))
        cpool = ctx.enter_context(tc.tile_pool(name="call", bufs=1))
        opool = ctx.enter_context(tc.tile_pool(name="outs", bufs=2))
        sqpool = ctx.enter_context(tc.tile_pool(name="sq", bufs=2))
        lwp = ctx.enter_context(tc.tile_pool(name="lw", bufs=1))
        pt_ps = ctx.enter_context(tc.tile_pool(name="ptp", bufs=2, space="PSUM"))
        h1_ps = ctx.enter_context(tc.tile_pool(name="h1p", bufs=2, space="PSUM"))
        h2_ps = ctx.enter_context(tc.tile_pool(name="h2p", bufs=2, space="PSUM"))
        c_ps = ctx.enter_context(tc.tile_pool(name="cp", bufs=2, space="PSUM"))

        # ---- z loads for group 0 go out first (SWDGE, casting f32->bf16);
        # weight DMAs ride HWDGE queues in parallel. The first two tiles
        # are column-chunked so the PE can start transposing early.
        zn_group = {g: [] for g in range(NG)}

        def load_group(g):
            for q in range(TPG):
                bt = TPG * g + q
                znt = zpool.tile([128, D], BF16, tag="zn", name="zn")
                ncks = 3 if bt <= 1 else 2
                for ck in range(ncks):
                    cs = ck * (D // ncks)
                    nc.gpsimd.dma_start(
                        znt[:, cs:cs + D // ncks],
                        z_d[bt * 128:(bt + 1) * 128, cs:cs + D // ncks],
                    )
                zn_group[g].append(znt)

        # t-row first: the h1p group openers depend on it
        te = const.tile([1, B], BF16, tag="te")
        nc.gpsimd.dma_start(te[:], tT_d[:])
        ident = const.tile([128, 128], BF16, tag="ident")
        nc.sync.dma_start(ident[:], id_d[:])
        w1s = const.tile([128, NCH, 128], BF16, tag="w1s")
        nc.sync.dma_start(w1s[:], w1m_d[:].rearrange("p (c h) -> p c h", c=NCH))
        w1e = const.tile([1, 128], BF16, tag="w1e")
        nc.sync.dma_start(w1e[:], w1e_d[:])
        b1c = const.tile([H, 1], F32, tag="b1c")
        nc.sync.dma_start(b1c[:], b1c_d[:])

        load_group(0)

        # needed only after mm1 of group 0 -- keep early HBM bandwidth for z
        w2 = const.tile([H, H], F32R, tag="w2")
        nc.sync.dma_start(w2[:], w2_d[:])
        b2c = const.tile([H, 1], F32, tag="b2c")
        nc.sync.dma_start(b2c[:], b2c_d[:])
        w3s = const.tile([H, CD], F32R, tag="w3s")
        nc.sync.dma_start(w3s[:], w3s_d[:])
        b3f = const.tile([128, CD], F32, tag="b3f")
        nc.sync.dma_start(b3f[:], b3f_d[:])

        load_group(1)

        c_all = cpool.tile([128, NBT, CD], F32, tag="c_all")
        s_parts = lwp.tile([128, NBT, 2], F32, tag="s_parts")

        def lambert_and_store(st, cnt):
            """Solve W for tiles [st, st+cnt) via asymptotic series + one
            log-Newton polish step, scale c by -k, DMA out."""
            def lt(nm):
                return lwp.tile([128, cnt], F32, tag=f"{nm}{st}", name=f"{nm}{st}")

            sv = lt("lw_sv")
            nc.vector.tensor_add(
                sv[:], s_parts[:, st:st + cnt, 0], s_parts[:, st:st + cnt, 1]
            )
            sv = sv[:]
            x = lt("lw_x")
            nc.vector.tensor_scalar(x[:], sv, 256.0, 8.0, ALU.mult, ALU.max)
            L1 = lt("lw_L1")
            nc.scalar.activation(L1[:], x[:], AF.Ln)
            L2 = lt("lw_L2")
            nc.scalar.activation(L2[:], L1[:], AF.Ln)
            # w = L1 - L2 + L2/L1 + L2*(L2-2)/(2*L1^2)
            r1 = lt("lw_r1")
            nc.vector.reciprocal(r1[:], L1[:])
            a = lt("lw_a")
            nc.vector.tensor_mul(a[:], L2[:], r1[:])
            w = lt("lw_w")
            nc.vector.tensor_sub(w[:], L1[:], L2[:])
            nc.vector.tensor_add(w[:], w[:], a[:])
            t = lt("lw_t")
            nc.vector.tensor_scalar(t[:], L2[:], -2.0, 0.5, ALU.add, ALU.mult)
            nc.vector.tensor_mul(t[:], t[:], a[:])
            nc.vector.tensor_mul(t[:], t[:], r1[:])
            nc.vector.tensor_add(w[:], w[:], t[:])
            # k = sqrt(256*w/s)  (0 when s == 0: w*rcp(s-guard) ~ 0);
            # the series alone is ~1e-4 accurate -- far below the bf16 floor
            sg = lt("lw_sg")
            rcp = lt("lw_rcp")
            nc.vector.tensor_scalar_max(sg[:], sv, 1e-30)
            nc.vector.reciprocal(rcp[:], sg[:])
            nc.vector.tensor_mul(sg[:], w[:], rcp[:])
            kpos = lt("lw_kpos")
            nc.scalar.activation(kpos[:], sg[:], AF.Sqrt, scale=256.0)
            for i in range(cnt):
                bt = st + i
                ot = opool.tile([128, CD], F32, tag="ot", name="ot")
                nc.vector.tensor_scalar(
                    ot[:], c_all[:, bt, :], kpos[:, i:i + 1], -1.0,
                    ALU.mult, ALU.mult,
                )
                nc.sync.dma_start(out_d[bt * 128:(bt + 1) * 128, :], ot[:])

        def emit_square(bt, nb):
            sq = sqpool.tile([128, 512], F32, tag="sq", name="sq")
            nc.scalar.activation(
                sq[:], c_all[:, bt, nb * 512:(nb + 1) * 512],
                AF.Square, accum_out=s_parts[:, bt, nb:nb + 1],
            )

        def tail_tile(bt, h1p):
            # per-tile tail (group 1): narrow chain, squares deferred
            h1s = hpool.tile([H, 128], F32R, tag="h1s", name="h1s")
            nc.scalar.activation(h1s[:], h1p[0:H, :], AF.Tanh, bias=b1c[:])
            h2p = h2_ps.tile([H, 128], F32, tag="h2p", name="h2p")
            nc.tensor.matmul(h2p[:], w2[:], h1s[:], start=True, stop=True)
            h2s = hpool.tile([H, 128], F32R, tag="h2s", name="h2s")
            nc.scalar.activation(h2s[:], h2p[:], AF.Tanh, bias=b2c[:])
            for nb in range(2):
                cp = c_ps.tile([128, 512], F32, tag="cp", name="cp")
                nc.tensor.matmul(
                    cp[:], h2s[:], w3s[:, nb * 512:(nb + 1) * 512],
                    start=True, stop=True,
                )
                nc.vector.tensor_add(
                    c_all[:, bt, nb * 512:(nb + 1) * 512],
                    cp[:], b3f[:, nb * 512:(nb + 1) * 512],
                )

        def tail_half(g, hf, h1p, defer_squares=False):
            # process quarters [2*hf, 2*hf+1] of group g (h1p is [128, 256])
            h1s = hpool.tile([H, 256], F32R, tag="h1s", name="h1s")
            nc.scalar.activation(h1s[:], h1p[0:H, :], AF.Tanh, bias=b1c[:])
            h2p = h2_ps.tile([H, 256], F32, tag="h2p", name="h2p")
            nc.tensor.matmul(h2p[:], w2[:], h1s[:], start=True, stop=True)
            h2s = hpool.tile([H, 256], F32R, tag="h2s", name="h2s")
            nc.scalar.activation(h2s[:], h2p[:], AF.Tanh, bias=b2c[:])
            for qq in range(2):
                bt = TPG * g + 2 * hf + qq
                for nb in range(2):
                    cp = c_ps.tile([128, 512], F32, tag="cp", name="cp")
                    nc.tensor.matmul(
                        cp[:], h2s[:, qq * 128:(qq + 1) * 128],
                        w3s[:, nb * 512:(nb + 1) * 512],
                        start=True, stop=True,
                    )
                    # c = cp + b3S  (DVE, PSUM -> SBUF), then the half's
                    # squared row-sum immediately (keeps squares off the tail)
                    nc.vector.tensor_add(
                        c_all[:, bt, nb * 512:(nb + 1) * 512],
                        cp[:], b3f[:, nb * 512:(nb + 1) * 512],
                    )
                    if not defer_squares:
                        emit_square(bt, nb)

        # ---- main loop: batch-tile-major z pipeline, half-group tails ----
        for g in range(NG):
            zn = zn_group[g]
            work = [(q, jg) for q in range(TPG) for jg in range(NJG)]
            h1ps = {}
            if g == 0:
                for hf in range(2):
                    h1ps[hf] = h1_ps.tile([128, 256], F32, tag="h1p", name="h1p")
                    # t column opens the half's accumulation group
                    cst = (g * TPG + 2 * hf) * 128
                    nc.tensor.matmul(
                        h1ps[hf][:], w1e[:], te[:, cst:cst + 256],
                        start=True, stop=False,
                    )
            pts = {}

            def emit_transpose(idx):
                q, jg = work[idx]
                pt = pt_ps.tile([128, 1024], BF16, tag="pt", name="pt")
                for u in range(8):
                    j = jg * 8 + u
                    nc.tensor.matmul(
                        pt[:, u * 128:(u + 1) * 128],
                        zn[q][:, j * 128:(j + 1) * 128],
                        ident[:],
                        start=(u == 0), stop=(u == 7),
                        is_transpose=True,
                    )
                pts[idx] = pt

            emit_transpose(0)
            for idx, (q, jg) in enumerate(work):
                if idx + 1 < len(work):
                    emit_transpose(idx + 1)  # keep PE one panel ahead
                zt = ztp.tile([128, 1024], BF16, tag="zt", name="zt")
                nc.vector.tensor_copy(zt[:], pts.pop(idx)[:])
                hf, qq = q // 2, q % 2
                lastq = (qq == 1 and jg == NJG - 1)
                bt = TPG * g + q
                if g == 0:
                    tgt = h1ps[hf][:, qq * 128:(qq + 1) * 128]
                    stop_now = (lastq and True)
                else:
                    if jg == 0:
                        h1ps[q] = h1_ps.tile(
                            [128, 128], F32, tag="h1p", name="h1p"
                        )
                        nc.tensor.matmul(
                            h1ps[q][:], w1e[:],
                            te[:, bt * 128:(bt + 1) * 128],
                            start=True, stop=False,
                        )
                    tgt = h1ps[q][:]
                    stop_now = (jg == NJG - 1)
                for u in range(8):
                    j = jg * 8 + u
                    nc.tensor.matmul(
                        tgt, w1s[:, j, :], zt[:, u * 128:(u + 1) * 128],
                        start=False, stop=(stop_now and u == 7),
                    )
                if g == 0:
                    if lastq:
                        tail_half(g, hf, h1ps.pop(hf))
                        if hf == 1:
                            lambert_and_store(0, TPG)
                else:
                    if jg == NJG - 1:
                        tail_tile(bt, h1ps.pop(q))
                        # squares deferred so later tiles' tanh stays ahead
                        # of them in the ACT FIFO
                        if q == TPG - 1:
                            for bt2 in (TPG, TPG + 1):
                                emit_square(bt2, 0)
                                emit_square(bt2, 1)
                            lambert_and_store(TPG, 2)
                            for bt2 in (TPG + 2, TPG + 3):
                                emit_square(bt2, 0)
                                emit_square(bt2, 1)
                            lambert_and_store(TPG + 2, 2)

    nc.compile()
    return nc


def host_prep(z, t, W1, b1, W2, b2, W3, b3):
    """Host-side weight re-layout + per-core shard maps."""
    f = np.float32
    bf = ml_dtypes.bfloat16
    z = np.asarray(z, f)
    t = np.asarray(t, f)
    W1 = np.asarray(W1, f)
    b1 = np.asarray(b1, f)
    W2 = np.asarray(W2, f)
    b2 = np.asarray(b2, f)
    W3 = np.asarray(W3, f)
    b3 = np.asarray(b3, f)

    # mm1 stationary chunks (bf16, padded to 128 cols for FWL):
    # w1m[p, j*128 + h] = W1[1 + j*128 + p, h]
    w1m = np.zeros((128, NCH, 128), bf)
    w1m[:, :, :H] = W1[1:, :].reshape(NCH, 128, H).transpose(1, 0, 2).astype(bf)
    w1m = np.ascontiguousarray(w1m.reshape(128, NCH * 128))
    w1e = np.zeros((1, 128), bf)
    w1e[0, :H] = W1[0, :].astype(bf)
    b1c = np.ascontiguousarray(b1.reshape(H, 1))
    b2c = np.ascontiguousarray(b2.reshape(H, 1))

    # fold the p -> c map into W3 (and b3)
    W3r = W3.reshape(H, CD // 4, 12)
    W3S = np.empty((H, CD // 4, 4), f)
    W3S[..., 0] = (W3r[..., 6] + W3r[..., 7] + W3r[..., 8]) / MASS
    W3S[..., 1] = W3r[..., 9]
    W3S[..., 2] = W3r[..., 10]
    W3S[..., 3] = W3r[..., 11]
    b3r = b3.reshape(CD // 4, 12)
    b3S = np.empty((CD // 4, 4), f)
    b3S[..., 0] = (b3r[..., 6] + b3r[..., 7] + b3r[..., 8]) / MASS
    b3S[..., 1] = b3r[..., 9]
    b3S[..., 2] = b3r[..., 10]
    b3S[..., 3] = b3r[..., 11]
    w3s = np.ascontiguousarray(W3S.reshape(H, CD))
    b3f = np.ascontiguousarray(np.broadcast_to(b3S.reshape(1, CD), (128, CD)))

    ident = np.eye(128, dtype=bf)

    in_maps = []
    for c in range(N_CORES):
        sl = slice(c * B, (c + 1) * B)
        in_maps.append({
            "z": np.ascontiguousarray(z[sl]),
            "tT": np.ascontiguousarray(t[sl].reshape(1, B)),
            "w1m": w1m,
            "w1e": w1e,
            "b1c": b1c,
            "w2": W2,
            "b2c": b2c,
            "w3s": w3s,
            "b3f": b3f,
            "ident": ident,
        })
    return in_maps


_NC_CACHE = None


def _get_nc():
    global _NC_CACHE
    if _NC_CACHE is None:
        _NC_CACHE = build_kernel()
    return _NC_CACHE


def run(inputs, trace=False):
    """Returns (full_output, BassKernelResults)."""
    nc = _get_nc()
    in_maps = host_prep(**inputs)
    res = run_bass_kernel_spmd(
        nc, in_maps, list(range(N_CORES)), trace=trace,
    )
    out = np.concatenate([r["out"] for r in res.results], axis=0)
    return out.astype(np.float32, copy=False), res


def kernel(**inputs):
    out, _ = run(inputs)
    return out
